# revision 17
# baseline (speedup 1.0000x reference)
"""Windowed attention + dynamic relative position bias on 8 NeuronCores.

Shapes: q,k,v [B=128, H=8, N=256, D=32] f32; pos-MLP width P=16; h=w=16.
Sharding: head-parallel - core c computes head c for all 128 batch windows;
the per-core head is selected purely by the w3 column passed to that core
(program is SPMD-identical).

v2 design (multi-engine exp, fp16 IO, host normalization):
  - All math in "z units": S = alpha*qk with alpha = 1024*log2(e)/sqrt(D),
    bias path in 1024*log2(e) units. exp(x) == 2^(Z/1024) for Z in z units.
  - Softmax exp is split across TWO engines per tile ([128,1024] of S):
      Act: activation Exp (scale=ln2/1024) -> fp16 E.
      DVE: custom fused op EXP2_BITS_ANT computing the BITS of fp16(2^z):
           Z=S+zbias; N=(Z+B)-B (magic round to 1024s); F=Z-N;
           u16 = trunc(C2*F*F + Z + C1).  One DVE instr per tile, bias
           fused via zbias (Src1).  Max rel err ~0.2%.
  - Bias application per Act tile: either PE fold (J-matmul accumulate of
    btrev into S) or post-exp multiply by expb (fp16; on DVE or Pool).
  - No on-device normalization: PV uses ones-augmented V; O and Z columns
    are DMA'd PSUM->DRAM with f32->fp16 conversion; host divides O/Z.
  - q/k/v are host-packed fp16 (q pre-scaled by alpha): halves DMA bytes.
  - Per-row exp scale constants differ per tile path but every softmax row
    lives inside one tile, so constants cancel in the host division.
  - DMA dispatch spread to respect the shared HWDGE generator: q chunks on
    Pool SWDGE, k/consts/outs on sync, v/gathers on the DVE queue.
"""

import os
import numpy as np

B, H, N, D = 128, 8, 256, 32
P = 16
NCORES = 8
NGROUPS = 16
NPRE = int(os.environ.get("K_NPRE", "6"))          # prefix groups before expb
DRAIN = int(os.environ.get("K_DRAIN", "2"))        # extra PVs per new half-group
CHUNKS = [(0, 1), (2, 5), (6, 10), (11, 15)]
CHUNK_OF_GROUP = [0, 0, 1, 1, 1, 1, 2, 2, 2, 2, 2, 3, 3, 3, 3, 3]

LOG2E = 1.4426950408889634
ALPHA = float(1024.0 * LOG2E / np.sqrt(D))         # q prescale (host)
ACT_SCALE = float(np.log(2.0) / 1024.0)            # Act exp scale in z units
W3_SCALE = float(1024.0 * LOG2E)                   # bias MLP output scale
BSHIFT = -6144.0                                   # -6 octaves, inside b3c
BMAGIC = float(1.5 * 2 ** 33)                      # fp32 round-to-1024 magic
EXP_C1 = 433.57                                    # mantissa-parabola const
EXP_C2 = 3.3007e-4                                 # mantissa-parabola curv
ZK_CONST = 12800.0                                 # prefix-DVE zbias (K only)
ZKF_CONST = 12800.0 + 6144.0                       # folded-DVE zbias const

# tile modes: 'A' = Act exp + later emul; 'F' = Act exp + PE fold;
#             'D' = DVE custom exp (fused bias); prefix 'D' uses ZK + emul.
X_CNT = int(os.environ.get("K_X", "21"))           # DVE-exp tiles (of 64)
F_CNT = int(os.environ.get("K_F", "10"))            # PE-fold tiles
MDVE_CNT = int(os.environ.get("K_MDVE", "5"))     # DVE-emul quota (rest Pool)
PRE_D = int(os.environ.get("K_PRED", "0"))         # prefix DVE-exp tiles
BRIDGE = int(os.environ.get("K_BRIDGE", "4"))      # unfolded tiles after prefix
GHW = int(os.environ.get("K_GHW", "10"))           # gathers on HWDGE (of 16)
EVA_ACT = int(os.environ.get("K_EVA", "13"))       # PSUM evacs on Act (of 32)

# merged const block: cbiga | cbigb | jmat
_CBA = {"bprojt": 0, "g1t": 128, "lb1t": 256, "linb1t": 384, "w1": 512}
_CBB = {"w2": 0, "ident": 128, "g2t": 256, "lb2t": 384, "linb2t": 512,
        "g3t": 640, "lb3t": 768, "w3c": 896}
CONSTWA = 640
CONSTWB = 904
CONSTW = CONSTWA + CONSTWB + 128                   # + jmat
JMAT_OFF = CONSTWA + CONSTWB

_BUILD_CACHE = {}


def _tile_modes():
    """Assign one of A/F/D to each of the 64 tiles, plus emul engine."""
    modes = [None] * 64
    npre_t = 4 * NPRE
    dleft = PRE_D
    for t in range(npre_t):
        if dleft > 0 and t % 2 == 1:
            modes[t] = "D"
            dleft -= 1
        else:
            modes[t] = "A"
    npost = 64 - npre_t
    d_post = X_CNT - (PRE_D - dleft)
    acc_d = acc_f = 0
    for i in range(npost):
        t = npre_t + i
        nd = ((i + 1) * d_post) // npost
        if nd > acc_d:
            modes[t] = "D"
            acc_d = nd
        else:
            nf = ((i + 1) * F_CNT) // npost
            if nf > acc_f and t >= npre_t + BRIDGE:
                modes[t] = "F"
                acc_f = nf
            else:
                modes[t] = "A"
    eng = {}
    for t in range(npre_t + BRIDGE):
        eng[t] = "dve"
    post_need = [t for t in range(npre_t + BRIDGE, 64) if modes[t] == "A"]
    nm = max(len(post_need), 1)
    for j, t in enumerate(post_need):
        if ((j + 1) * MDVE_CNT) // nm > (j * MDVE_CNT) // nm:
            eng[t] = "dve"
        else:
            eng[t] = "pool"
    return modes, eng


def _register_exp_op():
    if "op" in _BUILD_CACHE:
        return _BUILD_CACHE["op"]
    from concourse.dve_spec import Spec, Src0, Src1, C0, C1, C2, lower
    from concourse import dve_ops
    from concourse.dve_table_gen import dve_ver_for
    from concourse.dve_uop import DveOpSpec

    for o in dve_ops.OPS:
        if o.name == "EXP2_BITS_ANT":
            _BUILD_CACHE["op"] = o
            return o

    Z = Src0 + Src1
    Nq = (Z + C0) - C0
    F = Z - Nq
    body = (C2 * F) * F + (Z + C1)

    def ref(in0, in1, s0, s1, imm2):
        f32 = np.float32
        Zv = f32(f32(in0.astype(np.float32)) + f32(in1.astype(np.float32)))
        t = f32(Zv + f32(s0))
        Nv = f32(t - f32(s0))
        Fv = f32(Zv - Nv)
        u = f32(f32(f32(f32(imm2)) * Fv) * Fv + f32(Zv + f32(s1)))
        return np.clip(u, 0.0, 65535.0)

    spec = Spec(body=body, reference=ref)
    ver = dve_ver_for("TRN2")
    row = dve_ops._CUSTOM_DVE_ROW_BASE + len(dve_ops.OPS)
    sha = DveOpSpec(name="EXP2_BITS_ANT", opcode=row,
                    uops=lower(spec, ver=ver), rd1_en=True).sha(ver)
    op = dve_ops.DveOp("EXP2_BITS_ANT", spec, subdim=False,
                       uops_sha={ver: sha})
    dve_ops.OPS.append(op)
    dve_ops.CUSTOM_DVE_SPECS[op.name] = spec
    dve_ops._SUB_OPCODE_FOR_NAME[op.name] = row
    _BUILD_CACHE["op"] = op
    return op


def _build():
    if "nc" in _BUILD_CACHE:
        return _BUILD_CACHE["nc"]
    import concourse.bacc as bacc
    import concourse.mybir as mybir
    from concourse.tile import TileContext
    from bass_rust import AP

    exp_op = _register_exp_op()

    F32 = mybir.dt.float32
    F32R = mybir.dt.float32r
    FP16 = mybir.dt.float16
    U16 = mybir.dt.uint16
    AF = mybir.ActivationFunctionType
    AX = mybir.AxisListType
    ALU = mybir.AluOpType
    I32 = mybir.dt.int32

    nc = bacc.Bacc("TRN2", target_bir_lowering=False, debug=False,
                   num_devices=NCORES)

    # host-prearranged layouts (see build_in_maps), all fp16:
    # qd [128 p=(bi,d), (g 16, hh 2, n 256)] fp16, pre-scaled by ALPHA
    # kd [128 p=(bi,d), (g 16, hh 2, mb 2, m 128)] fp16
    # vd [128 p=m, (b 128, c 2, e 33)] fp16 (e==32 -> 1.0)
    qd = nc.dram_tensor("qd", [128, 8192], FP16, kind="ExternalInput")
    kd = nc.dram_tensor("kd", [128, 8192], FP16, kind="ExternalInput")
    vd = nc.dram_tensor("vd", [128, 8448], FP16, kind="ExternalInput")
    biasd_d = nc.dram_tensor("biasd", [2, 1040], F32, kind="ExternalInput")
    b3c_d = nc.dram_tensor("b3c", [8, 1], F32, kind="ExternalInput")
    cbig_d = nc.dram_tensor("cbig", [128, CONSTW], F32, kind="ExternalInput")

    posd = nc.dram_tensor("posd", [1, 1024], F32R, kind="Internal")
    # raw O (32 cols) + Z (1 col) per j, 8 j per half-group, fp16
    out_d = nc.dram_tensor("out", [128, 8448], FP16, kind="ExternalOutput")

    MODES, EMUL_ENG = _tile_modes()

    with TileContext(nc) as tc:
        with (
            tc.tile_pool(name="const", bufs=1) as constp,
            tc.tile_pool(name="vpool", bufs=1) as vpool,
            tc.tile_pool(name="mlp", bufs=2) as mlpp,
            tc.tile_pool(name="epool", bufs=int(os.environ.get("K_EP", "28"))) as epool,
            tc.tile_pool(name="spsum", bufs=int(os.environ.get("K_SB", "3")), space="PSUM") as spsum,
            tc.tile_pool(name="auxpsum", bufs=int(os.environ.get("K_AB", "2")), space="PSUM") as auxpsum,
        ):
            # ---- full-size q/k/v SBUF tiles; chunked loads emitted lazily
            q_all = vpool.tile([128, 8192], FP16)
            k_all = vpool.tile([128, 8192], FP16)
            v_all = vpool.tile([128, 8448], FP16)

            chunk_loaded = [False] * len(CHUNKS)

            def emit_chunk(ci):
                g0, g1 = CHUNKS[ci]
                ng = g1 - g0 + 1
                qk0 = g0
                if ci == 0:
                    qk0 = 1      # group 0 of q/k loaded via the fast path
                # q on Pool SWDGE; k on sync; v on the DVE queue (all three
                # dispatchers run concurrently; HWDGE generator is shared)
                nc.gpsimd.dma_start(
                    q_all[:, 512 * qk0:512 * (g1 + 1)],
                    AP(qd, 512 * qk0,
                       [[8192, 128], [1, 512 * (g1 - qk0 + 1)]]))
                nc.sync.dma_start(
                    k_all[:, 512 * qk0:512 * (g1 + 1)],
                    AP(kd, 512 * qk0,
                       [[8192, 128], [1, 512 * (g1 - qk0 + 1)]]))
                nc.scalar.dma_start(
                    v_all[:, 528 * g0:528 * (g1 + 1)],
                    AP(vd, 528 * g0, [[8448, 128], [1, 528 * ng]]))

            def ensure_chunk(ci):
                if not chunk_loaded[ci]:
                    chunk_loaded[ci] = True
                    emit_chunk(ci)

            # fast path for the very first QK: k via sync HWDGE, q via Pool
            # SWDGE - different dispatchers run concurrently
            nc.sync.dma_start(k_all[:, 0:512],
                              AP(kd, 0, [[8192, 128], [1, 512]]))
            nc.gpsimd.dma_start(q_all[:, 0:512],
                                AP(qd, 0, [[8192, 128], [1, 512]]))

            ensure_chunk(0)
            ensure_chunk(1)

            biasd = constp.tile([2, 1040], F32)
            nc.sync.dma_start(biasd[:, :], biasd_d[:, :])
            cbig = constp.tile([128, CONSTW], F32)
            nc.sync.dma_start(cbig[:, 0:CONSTWA], cbig_d[:, 0:CONSTWA])
            nc.sync.dma_start(cbig[:, CONSTWA:], cbig_d[:, CONSTWA:])
            b3c = constp.tile([8, 1], F32)
            nc.sync.dma_start(b3c[:, :], b3c_d[:, :])
            jmat_r = constp.tile([128, 128], F32R)
            nc.vector.tensor_copy(jmat_r[:, :],
                                  cbig[:, JMAT_OFF:JMAT_OFF + 128])
            magic_t = constp.tile([128, 8], I32)
            nc.vector.memset(magic_t[:, :], 0x5F3759DF)
            # constant zbias tiles: zk for prefix DVE-exp (bias applied
            # later via expb), zkf for post-prefix DVE-exp (bias PE-folded
            # into S; btrev carries -6144 so the const re-centers the phase)
            zk = constp.tile([128, 1024], F32)
            nc.gpsimd.memset(zk[:, :], ZK_CONST)
            zkf = constp.tile([128, 1024], F32)
            nc.gpsimd.memset(zkf[:, :], ZKF_CONST)

            def cb(nm):
                if nm in _CBA:
                    o = _CBA[nm]
                else:
                    o = CONSTWA + _CBB[nm]
                w = 8 if nm == "w3c" else 128
                return cbig[:, o:o + w]

            mlp_env = {}

            def _mlp_layer(x_sb, g_t, beta_t, w_t, linb_t, last=False):
                x3 = x_sb[:, :].rearrange("p (j f) -> p j f", f=16)
                mz = mlpp.tile([128, 8], F32, tag="mz")
                nc.vector.tensor_reduce(mz[:, :], x3, AX.X, ALU.add)
                xc = mlpp.tile([128, 128], F32, tag="xc")
                xc3 = xc[:, :].rearrange("p (j f) -> p j f", f=16)
                # xc' = mz/16 - x  (negated; g tiles are host-negated)
                nc.vector.scalar_tensor_tensor(
                    xc3, mz[:, :].unsqueeze(2).broadcast_to((128, 8, 16)),
                    1.0 / 16.0, x3, ALU.mult, ALU.subtract)
                sq = mlpp.tile([128, 128], F32, tag="sq")
                sq3 = sq[:, :].rearrange("p (j f) -> p j f", f=16)
                nc.vector.tensor_mul(sq3, xc3, xc3)
                vz = mlpp.tile([128, 8], F32, tag="vz")
                nc.vector.tensor_reduce(vz[:, :], sq3, AX.X, ALU.add)
                # rsqrt(v/16 + eps) fully on DVE (bit-magic + 1 Newton
                # step) so ScalarE only ever runs Exp (one act table).
                w = mlpp.tile([128, 8], F32, tag="w")
                nc.vector.tensor_scalar(w[:, :], vz[:, :], 1.0 / 16.0, 1e-5,
                                        ALU.mult, ALU.add)
                sh = mlpp.tile([128, 8], I32, tag="sh")
                nc.vector.tensor_single_scalar(sh[:, :],
                                               w[:, :].bitcast(I32), 1,
                                               ALU.arith_shift_right)
                yi = mlpp.tile([128, 8], I32, tag="yi")
                nc.vector.tensor_sub(yi[:, :], magic_t[:, :], sh[:, :])
                y0 = yi[:, :].bitcast(F32)
                rz = None
                for it in range(int(os.environ.get("K_NEWTON", "1"))):
                    t = mlpp.tile([128, 8], F32, tag=f"nt{it}")
                    nc.vector.tensor_mul(t[:, :], w[:, :], y0)
                    nc.vector.tensor_mul(t[:, :], t[:, :], y0)
                    nc.vector.tensor_scalar(t[:, :], t[:, :], -0.5, 1.5,
                                            ALU.mult, ALU.add)
                    y1 = mlpp.tile([128, 8], F32, tag=f"ny{it}")
                    nc.vector.tensor_mul(y1[:, :], y0, t[:, :])
                    y0 = y1[:, :]
                    rz = y1
                xn = mlpp.tile([128, 128], F32, tag="xn")
                xn3 = xn[:, :].rearrange("p (j f) -> p j f", f=16)
                nc.vector.tensor_mul(
                    xn3, xc3, rz[:, :].unsqueeze(2).broadcast_to((128, 8, 16)))
                y = mlpp.tile([128, 128], F32, tag="y")
                nc.vector.tensor_mul(y[:, :], xn[:, :], g_t[:, :])
                # +beta folded into the transpose (PSUM accumulate of the
                # column-broadcast betaT const); relu folded into the evac
                pt = auxpsum.tile([128, 512], F32, tag="aux2")
                nc.tensor.matmul(pt[:, :128], y[:, :], cb("ident"),
                                 is_transpose=True, start=True, stop=False)
                nc.tensor.matmul(pt[:, :128], cb("ident"), beta_t,
                                 start=False, stop=True)
                yT = mlpp.tile([128, 128], F32, tag="yT")
                nc.vector.tensor_scalar_max(yT[:, :], pt[:, :128], 0.0)
                if last:
                    return yT
                px = auxpsum.tile([128, 512], F32, tag="aux2")
                nc.tensor.matmul(px[:, :128], yT[:, :], w_t)
                xnext = mlpp.tile([128, 128], F32, tag="xnext")
                nc.vector.tensor_add(xnext[:, :], px[:, :128], linb_t)
                return xnext

            def emit_mlp_stage(stage):
                """0=x0, 1..3=LN layers, 4=pos->DRAM->gather (sets btrev)."""
                env = mlp_env
                if stage == 0:
                    px0 = auxpsum.tile([128, 512], F32, tag="aux2")
                    for j in range(8):
                        nc.tensor.matmul(px0[:, 16 * j:16 * j + 16],
                                         biasd[:, 128 * j:128 * j + 128],
                                         biasd[:, 1024:1040])
                    x0 = mlpp.tile([128, 128], F32, tag="x0")
                    nc.vector.tensor_add(x0[:, :], px0[:, :128], cb("bprojt"))
                    env["x0"] = x0
                    return
                if stage == 1:
                    env["x1"] = _mlp_layer(env["x0"], cb("g1t"), cb("lb1t"),
                                           cb("w1"), cb("linb1t"))
                    return
                if stage == 2:
                    env["x2"] = _mlp_layer(env["x1"], cb("g2t"), cb("lb2t"),
                                           cb("w2"), cb("linb2t"))
                    return
                if stage == 3:
                    env["y3T"] = _mlp_layer(env["x2"], cb("g3t"), cb("lb3t"),
                                            None, None, last=True)
                    return
                # stage 4: posT -> DRAM -> Toeplitz gather (reversed m)
                pos_ps = auxpsum.tile([128, 512], F32, tag="aux2")
                nc.tensor.matmul(pos_ps[0:8, :128], cb("w3c"), env["y3T"][:, :])
                pos_sb = constp.tile([8, 128], F32R)
                nc.vector.tensor_scalar_add(pos_sb[:, :], pos_ps[0:8, :128],
                                            b3c[:, 0:1])
                nc.sync.dma_start(AP(posd, 0, [[128, 8], [1, 128]]),
                                  pos_sb[:, :])
                # btrev as one [128, (mbp 2, c 16, e 16)] tile; 8 DMAs of
                # 16 partitions each, 4D source APs, split across HWDGE
                # (sync/vector) and Pool SWDGE dispatchers
                bt = constp.tile([128, 512], F32R)
                gi = 0
                for mbp in range(2):
                    for a in range(8):
                        src = AP(posd, 31 * (8 * mbp + a),
                                 [[1, 16], [31, 16], [1, 16]])
                        dst = bt[16 * a:16 * a + 16,
                                 256 * mbp:256 * mbp + 256].rearrange(
                            "b (c e) -> b c e", e=16)
                        if gi % 16 < GHW:
                            if gi % 2 == 0:
                                nc.sync.dma_start(dst, src)
                            else:
                                nc.scalar.dma_start(dst, src)
                        else:
                            nc.gpsimd.dma_start(dst, src)
                        gi += 1
                env["btrev"] = [bt[:, 0:256], bt[:, 256:512]]

            def emit_expb():
                """expb (fp16, 2^(bias-6)) from btrev."""
                btrev = mlp_env["btrev"]
                pe_ = auxpsum.tile([128, 512], F32, tag="aux2", name="pexpb")
                for mb in range(2):
                    nc.tensor.matmul(pe_[:, 256 * mb:256 * mb + 256],
                                     jmat_r[:, :], btrev[1 - mb])
                expb = constp.tile([128, 512], FP16)
                nc.scalar.activation(expb[:, :], pe_[:, :512], AF.Exp,
                                     scale=ACT_SCALE)
                return expb

            # --- main pipeline over 64 tiles (2 per half-group) ---
            def emit_qk_exp(g, hh, split_exp=False):
                """QK matmuls + exp for half-group (g, hh) -> epair, emuls."""
                ho = 512 * g + 256 * hh
                epair = []
                emuls = []
                for half in range(2):
                    t_idx = 4 * g + 2 * hh + half
                    mode = MODES[t_idx]
                    in_prefix = t_idx < 4 * NPRE + BRIDGE
                    fold = mode == "F" or (mode == "D" and not in_prefix)
                    sp = spsum.tile([128, 1024], F32, tag="S",
                                    name=f"s{g}_{hh}_{half}")
                    for bi2 in range(2):
                        bi = 2 * half + bi2
                        fo = 512 * bi2
                        for mb in range(2):
                            out_ap = sp[:, fo + 256 * mb:fo + 256 * mb + 256]
                            nc.tensor.matmul(
                                out_ap,
                                k_all[32 * bi:32 * bi + 32,
                                      ho + 128 * mb:ho + 128 * mb + 128],
                                q_all[32 * bi:32 * bi + 32, ho:ho + 256],
                                tile_position=(32 * bi, 0),
                                start=True, stop=not fold)
                            if fold:
                                nc.tensor.matmul(
                                    out_ap, jmat_r[:, :],
                                    mlp_env["btrev"][1 - mb],
                                    tile_position=(0, 0),
                                    start=False, stop=True)
                    e = epool.tile([128, 1024], FP16, tag="E",
                                   name=f"e{g}_{hh}_{half}")
                    if mode == "D":
                        zt = zk if in_prefix else zkf
                        nc.vector._custom_dve(
                            exp_op, out=e[:, :].bitcast(U16),
                            in0=sp[:, :], in1=zt[:, :],
                            s0=BMAGIC, s1=EXP_C1, imm2=EXP_C2)
                        if in_prefix:
                            emuls.append((half, EMUL_ENG[t_idx]))
                    else:
                        if split_exp:
                            nc.scalar.activation(e[:, :512], sp[:, :512],
                                                 AF.Exp, scale=ACT_SCALE)
                            nc.scalar.activation(e[:, 512:], sp[:, 512:],
                                                 AF.Exp, scale=ACT_SCALE)
                        else:
                            nc.scalar.activation(e[:, :], sp[:, :], AF.Exp,
                                                 scale=ACT_SCALE)
                        if mode == "A":
                            emuls.append((half, EMUL_ENG[t_idx]))
                    epair.append(e)
                return epair, emuls

            def emit_emul(expb, epair, emuls):
                for half, eng in emuls:
                    e = epair[half]
                    e4 = e[:, :].rearrange("p (j mb n) -> p j mb n",
                                           mb=2, n=256)
                    bb = (expb[:, :].rearrange("p (mb n) -> p mb n", n=256)
                          .unsqueeze(1).broadcast_to((128, 2, 2, 256)))
                    if eng == "pool":
                        nc.gpsimd.tensor_mul(e4, e4, bb)
                    else:
                        nc.vector.tensor_mul(e4, e4, bb)

            evac_ctr = [0]

            def emit_pv(g, hh, epair):
                o_ps = auxpsum.tile([128, 264], F32, tag="aux2",
                                    name=f"ops{g}_{hh}")
                for bi in range(4):
                    e = epair[bi // 2]
                    fo = 512 * (bi % 2)
                    vb = 66 * (8 * g + 4 * hh + bi)
                    for nb in range(2):
                        j = 2 * bi + nb
                        for c in range(2):
                            nc.tensor.matmul(
                                o_ps[:, 33 * j:33 * j + 33],
                                e[:, fo + 256 * c + 128 * nb:
                                  fo + 256 * c + 128 * nb + 128],
                                v_all[:, vb + 33 * c:vb + 33 * c + 33],
                                start=(c == 0), stop=(c == 1))
                return o_ps

            def emit_evac_store(g, hh, o_ps, split=False):
                osb = epool.tile([128, 264], FP16, tag="osb",
                                 name=f"osb{g}_{hh}")

                def one(j0, nj):
                    src_ = o_ps[:, 33 * j0:33 * (j0 + nj)]
                    dst_ = osb[:, 33 * j0:33 * (j0 + nj)]
                    i = evac_ctr[0]
                    evac_ctr[0] += 1
                    if ((i + 1) * EVA_ACT) // 32 > (i * EVA_ACT) // 32:
                        nc.scalar.activation(dst_, src_, AF.Copy)
                    else:
                        nc.vector.tensor_copy(dst_, src_)
                    nc.sync.dma_start(
                        AP(out_d, 264 * (2 * g + hh) + 33 * j0,
                           [[8448, 128], [1, 33 * nj]]),
                        osb[:, 33 * j0:33 * (j0 + nj)])

                if split:
                    one(0, 4)
                    one(4, 4)
                else:
                    one(0, 8)

            # ---- schedule ----
            # Explicit stage lags over half-groups: at hg k the loop emits
            # evac+store(k-3), PV(k-2), emul(k-1), then QK+exp(k), oldest
            # first so each engine's in-order queue sees deps long
            # satisfied.  Prefix only QK+exps (+MLP); backlog drains at
            # DRAIN extra items per stage per new half-group.
            unmul = []   # (g, hh, epair, emuls)  exp'd, bias-mul pending
            unpv = []    # (g, hh, epair)         biased, PV pending
            unev = []    # (g, hh, o_ps)          PV'd, evac+store pending

            def step_evac(n):
                for _ in range(n):
                    if unev:
                        emit_evac_store(*unev.pop(0))

            def step_pv(n):
                for _ in range(n):
                    if unpv:
                        g_, hh_, ep_ = unpv.pop(0)
                        unev.append((g_, hh_, emit_pv(g_, hh_, ep_)))

            def step_emul(n, expb):
                for _ in range(n):
                    if unmul:
                        g_, hh_, ep_, em_ = unmul.pop(0)
                        emit_emul(expb, ep_, em_)
                        unpv.append((g_, hh_, ep_))

            stage_after = {1: 1, 2: 2, 3: 3, 4: 4}  # halfgroup -> mlp stage
            hg = 0
            emit_mlp_stage(0)
            for g in range(NPRE):
                ensure_chunk(CHUNK_OF_GROUP[min(g + 2, NGROUPS - 1)])
                for hh in range(2):
                    ep, em = emit_qk_exp(g, hh, split_exp=(g == 0))
                    unmul.append((g, hh, ep, em))
                    hg += 1
                    st = stage_after.get(hg)
                    if st == 4:
                        # keep Pool's queue ahead of the gathers
                        ensure_chunk(CHUNK_OF_GROUP[4])
                    if st is not None:
                        emit_mlp_stage(st)
            expb = emit_expb()

            CAP = int(os.environ.get("K_CAP", "3"))
            for g in range(NPRE, NGROUPS):
                ensure_chunk(CHUNK_OF_GROUP[min(g + 2, NGROUPS - 1)])
                for hh in range(2):
                    step_evac(min(CAP, max(0, len(unev) - 1)))
                    step_pv(min(CAP, max(0, len(unpv) - 1)))
                    step_emul(min(CAP, max(0, len(unmul) - 1)), expb)
                    ep, em = emit_qk_exp(g, hh)
                    unmul.append((g, hh, ep, em))
            # drain the tail: oldest-first except the final half-group's
            # chain, which jumps the queue (split store for overlap)
            while unmul or unpv or unev:
                step_evac(1)
                step_pv(1)
                step_emul(2, expb)
                if not unmul:
                    step_pv(2)
                    step_evac(2)

    nc.compile()
    _BUILD_CACHE["nc"] = nc
    return nc


def _host_constants():
    hh, ww = 16, 16
    bh, bw = np.meshgrid(np.arange(1 - hh, hh), np.arange(1 - ww, ww),
                         indexing="ij")
    biases = np.stack([bh, bw], -1).reshape(-1, 2).astype(np.float32)
    biasesT = np.zeros((2, 1024), np.float32)
    biasesT[:, :961] = biases.T
    return biasesT


def _blk8(w16):
    cout = w16.shape[1]
    blk = np.zeros((128, 8 * cout), np.float32)
    for j in range(8):
        blk[16 * j:16 * j + 16, cout * j:cout * j + cout] = w16
    return np.ascontiguousarray(blk)


def _tile16(vec):
    return np.ascontiguousarray(
        np.tile(np.asarray(vec, np.float32), (128, 8)))


def _tile16T(vec):
    # column-broadcast (transposed-space) tiling: out[16j+f, r] = vec[f]
    col = np.tile(np.asarray(vec, np.float32), 8)[:, None]
    return np.ascontiguousarray(np.tile(col, (1, 128)))


def build_in_maps(inputs):
    q = np.asarray(inputs["q"], np.float32)
    k = np.asarray(inputs["k"], np.float32)
    v = np.asarray(inputs["v"], np.float32)
    hh = int(np.asarray(inputs["h"]))
    ww = int(np.asarray(inputs["w"]))
    assert hh == 16 and ww == 16, (hh, ww)
    f32 = lambda name: np.asarray(inputs[name], np.float32)
    w3 = f32("w3")
    b3 = f32("b3")

    cblk = {
        "w1": _blk8(f32("w1")), "w2": _blk8(f32("w2")),
        "ident": np.eye(128, dtype=np.float32),
        "bprojt": _tile16(f32("b_proj")),
        "g1t": -_tile16(f32("ln1_g")), "lb1t": _tile16T(f32("ln1_b")),
        "linb1t": _tile16(f32("b1")),
        "g2t": -_tile16(f32("ln2_g")), "lb2t": _tile16T(f32("ln2_b")),
        "linb2t": _tile16(f32("b2")),
        "g3t": -_tile16(f32("ln3_g")), "lb3t": _tile16T(f32("ln3_b")),
    }
    biasd = np.zeros((2, 1040), np.float32)
    biasd[:, 0:1024] = _host_constants()
    biasd[:, 1024:1040] = f32("w_proj")

    def q_layout(x):
        # [128 w, 256 n, 32 d] -> [128 p=(bi,d), (g, hh, n)] fp16, *ALPHA
        x5 = (x * np.float32(ALPHA)).reshape(16, 2, 4, 256, 32)
        return np.ascontiguousarray(
            x5.transpose(2, 4, 0, 1, 3).reshape(128, 8192).astype(np.float16))

    def k_layout(x):
        # [128 w, 256 m, 32 d] -> [128 p=(bi,d), (g, hh, mb, m)] fp16
        x6 = x.reshape(16, 2, 4, 2, 128, 32)        # g hh bi mb m d
        return np.ascontiguousarray(
            x6.transpose(2, 5, 0, 1, 3, 4).reshape(128, 8192)
            .astype(np.float16))

    def v_layout(x):
        # [128 p=m, (b 128, c 2, e 33)] fp16; e==32 -> 1.0
        v4 = x.reshape(128, 2, 128, 32)             # b c p e
        out = np.ones((128, 128, 2, 33), np.float32)
        out[:, :, :, :32] = v4.transpose(2, 0, 1, 3)
        return np.ascontiguousarray(out.reshape(128, 8448).astype(np.float16))

    in_maps = []
    for c in range(NCORES):
        cbig = np.empty((128, CONSTW), np.float32)
        for nm, off in _CBA.items():
            cbig[:, off:off + 128] = cblk[nm]
        for nm, off in _CBB.items():
            if nm == "w3c":
                cbig[:, CONSTWA + off:CONSTWA + off + 8] = _blk8(
                    w3[:, c:c + 1] * np.float32(W3_SCALE))
            else:
                cbig[:, CONSTWA + off:CONSTWA + off + 128] = cblk[nm]
        cbig[:, JMAT_OFF:JMAT_OFF + 128] = np.eye(128, dtype=np.float32)[::-1]
        m = {
            "biasd": biasd,
            "cbig": np.ascontiguousarray(cbig),
            "qd": q_layout(q[:, c]),
            "kd": k_layout(k[:, c]),
            "vd": v_layout(v[:, c]),
            "b3c": np.full((8, 1),
                           b3[c] * np.float32(W3_SCALE) + np.float32(BSHIFT),
                           np.float32),
        }
        in_maps.append(m)
    return in_maps


def unshard_out(raw):
    # raw [128 p, (g 16, hh 2, j 8, e 33)] fp16 -> [B, N, D] f32 (normalize)
    r5 = raw.reshape(128, 16, 2, 8, 33).astype(np.float32)  # p g hh j e
    O = r5[..., :32]
    Z = r5[..., 32]
    out = O / Z[..., None]
    # b = (g*2+hh)*4 + j//2 ; n = (j%2)*128 + p ; d = e
    o6 = out.reshape(128, 16, 2, 4, 2, 32)           # p g hh bi nb e
    return np.ascontiguousarray(
        o6.transpose(1, 2, 3, 4, 0, 5).reshape(128, 256, 32))


def kernel(**inputs):
    from concourse.bass_utils import run_bass_kernel_spmd

    nc = _build()
    in_maps = build_in_maps(inputs)
    res = run_bass_kernel_spmd(nc, in_maps, core_ids=list(range(NCORES)))
    out = np.empty((B, H, N, D), np.float32)
    for c in range(NCORES):
        out[:, c] = unshard_out(res.results[c]["out"])
    return out


# revision 18
# speedup vs baseline: 1.0459x; 1.0459x over previous
"""Windowed attention + dynamic relative position bias on 8 NeuronCores.

Shapes: q,k,v [B=128, H=8, N=256, D=32] f32; pos-MLP width P=16; h=w=16.
Sharding: head-parallel - core c computes head c for all 128 batch windows;
the per-core head is selected purely by the w3 column passed to that core
(program is SPMD-identical).

v2 design (multi-engine exp, fp16 IO, host normalization):
  - All math in "z units": S = alpha*qk with alpha = 1024*log2(e)/sqrt(D),
    bias path in 1024*log2(e) units. exp(x) == 2^(Z/1024) for Z in z units.
  - Softmax exp is split across TWO engines per tile ([128,1024] of S):
      Act: activation Exp (scale=ln2/1024) -> fp16 E.
      DVE: custom fused op EXP2_BITS_ANT computing the BITS of fp16(2^z):
           Z=S+zbias; N=(Z+B)-B (magic round to 1024s); F=Z-N;
           u16 = trunc(C2*F*F + Z + C1).  One DVE instr per tile, bias
           fused via zbias (Src1).  Max rel err ~0.2%.
  - Bias application per Act tile: either PE fold (J-matmul accumulate of
    btrev into S) or post-exp multiply by expb (fp16; on DVE or Pool).
  - No on-device normalization: PV uses ones-augmented V; O and Z columns
    are DMA'd PSUM->DRAM with f32->fp16 conversion; host divides O/Z.
  - q/k/v are host-packed fp16 (q pre-scaled by alpha): halves DMA bytes.
  - Per-row exp scale constants differ per tile path but every softmax row
    lives inside one tile, so constants cancel in the host division.
  - DMA dispatch spread to respect the shared HWDGE generator: q chunks on
    Pool SWDGE, k/consts/outs on sync, v/gathers on the DVE queue.
"""

import os
import numpy as np

B, H, N, D = 128, 8, 256, 32
P = 16
NCORES = 8
NGROUPS = 16
NPRE = int(os.environ.get("K_NPRE", "4"))          # prefix groups before expb
DRAIN = int(os.environ.get("K_DRAIN", "2"))        # extra PVs per new half-group
CHUNKS = [(0, 1), (2, 5), (6, 10), (11, 15)]
CHUNK_OF_GROUP = [0, 0, 1, 1, 1, 1, 2, 2, 2, 2, 2, 3, 3, 3, 3, 3]

LOG2E = 1.4426950408889634
ALPHA = float(1024.0 * LOG2E / np.sqrt(D))         # q prescale (host)
ACT_SCALE = float(np.log(2.0) / 1024.0)            # Act exp scale in z units
W3_SCALE = float(1024.0 * LOG2E)                   # bias MLP output scale
BSHIFT = -6144.0                                   # -6 octaves, inside b3c
BMAGIC = float(1.5 * 2 ** 33)                      # fp32 round-to-1024 magic
EXP_C1 = 433.57                                    # mantissa-parabola const
EXP_C2 = 3.3007e-4                                 # mantissa-parabola curv
ZK_CONST = 12800.0                                 # prefix-DVE zbias (K only)
ZKF_CONST = 12800.0 + 6144.0                       # folded-DVE zbias const

# tile modes: 'A' = Act exp + later emul; 'F' = Act exp + PE fold;
#             'D' = DVE custom exp (fused bias); prefix 'D' uses ZK + emul.
X_CNT = int(os.environ.get("K_X", "18"))           # DVE-exp tiles (of 64)
F_CNT = int(os.environ.get("K_F", "12"))            # PE-fold tiles
MDVE_CNT = int(os.environ.get("K_MDVE", "19"))     # DVE-emul quota (rest Pool)
PRE_D = int(os.environ.get("K_PRED", "0"))         # prefix DVE-exp tiles
BRIDGE = int(os.environ.get("K_BRIDGE", "4"))      # unfolded tiles after prefix
GHW = int(os.environ.get("K_GHW", "10"))           # gathers on HWDGE (of 16)
EVA_ACT = int(os.environ.get("K_EVA", "13"))       # PSUM evacs on Act (of 32)

# merged const block: cbiga | cbigb | jmat
_CBA = {"bprojt": 0, "g1t": 128, "lb1t": 256, "linb1t": 384, "w1": 512}
_CBB = {"w2": 0, "ident": 128, "g2t": 256, "lb2t": 384, "linb2t": 512,
        "g3t": 640, "lb3t": 768, "w3c": 896}
CONSTWA = 640
CONSTWB = 904
CONSTW = CONSTWA + CONSTWB + 128                   # + jmat
JMAT_OFF = CONSTWA + CONSTWB

_BUILD_CACHE = {}


def _tile_modes():
    """Assign one of A/F/D to each of the 64 tiles, plus emul engine."""
    modes = [None] * 64
    npre_t = 4 * NPRE
    dleft = PRE_D
    for t in range(npre_t):
        if dleft > 0 and t % 2 == 1:
            modes[t] = "D"
            dleft -= 1
        else:
            modes[t] = "A"
    npost = 64 - npre_t
    d_post = X_CNT - (PRE_D - dleft)
    acc_d = acc_f = 0
    for i in range(npost):
        t = npre_t + i
        nd = ((i + 1) * d_post) // npost
        if nd > acc_d:
            modes[t] = "D"
            acc_d = nd
        else:
            nf = ((i + 1) * F_CNT) // npost
            if nf > acc_f and t >= npre_t + BRIDGE:
                modes[t] = "F"
                acc_f = nf
            else:
                modes[t] = "A"
    eng = {}
    for t in range(npre_t + BRIDGE):
        eng[t] = "dve"
    post_need = [t for t in range(npre_t + BRIDGE, 64) if modes[t] == "A"]
    nm = max(len(post_need), 1)
    for j, t in enumerate(post_need):
        if ((j + 1) * MDVE_CNT) // nm > (j * MDVE_CNT) // nm:
            eng[t] = "dve"
        else:
            eng[t] = "pool"
    return modes, eng


def _register_exp_op():
    if "op" in _BUILD_CACHE:
        return _BUILD_CACHE["op"]
    from concourse.dve_spec import Spec, Src0, Src1, C0, C1, C2, lower
    from concourse import dve_ops
    from concourse.dve_table_gen import dve_ver_for
    from concourse.dve_uop import DveOpSpec

    for o in dve_ops.OPS:
        if o.name == "EXP2_BITS_ANT":
            _BUILD_CACHE["op"] = o
            return o

    Z = Src0 + Src1
    Nq = (Z + C0) - C0
    F = Z - Nq
    body = (C2 * F) * F + (Z + C1)

    def ref(in0, in1, s0, s1, imm2):
        f32 = np.float32
        Zv = f32(f32(in0.astype(np.float32)) + f32(in1.astype(np.float32)))
        t = f32(Zv + f32(s0))
        Nv = f32(t - f32(s0))
        Fv = f32(Zv - Nv)
        u = f32(f32(f32(f32(imm2)) * Fv) * Fv + f32(Zv + f32(s1)))
        return np.clip(u, 0.0, 65535.0)

    spec = Spec(body=body, reference=ref)
    ver = dve_ver_for("TRN2")
    row = dve_ops._CUSTOM_DVE_ROW_BASE + len(dve_ops.OPS)
    sha = DveOpSpec(name="EXP2_BITS_ANT", opcode=row,
                    uops=lower(spec, ver=ver), rd1_en=True).sha(ver)
    op = dve_ops.DveOp("EXP2_BITS_ANT", spec, subdim=False,
                       uops_sha={ver: sha})
    dve_ops.OPS.append(op)
    dve_ops.CUSTOM_DVE_SPECS[op.name] = spec
    dve_ops._SUB_OPCODE_FOR_NAME[op.name] = row
    _BUILD_CACHE["op"] = op
    return op


def _build():
    if "nc" in _BUILD_CACHE:
        return _BUILD_CACHE["nc"]
    import concourse.bacc as bacc
    import concourse.mybir as mybir
    from concourse.tile import TileContext
    from bass_rust import AP

    exp_op = _register_exp_op()

    F32 = mybir.dt.float32
    F32R = mybir.dt.float32r
    FP16 = mybir.dt.float16
    U16 = mybir.dt.uint16
    AF = mybir.ActivationFunctionType
    AX = mybir.AxisListType
    ALU = mybir.AluOpType
    I32 = mybir.dt.int32

    nc = bacc.Bacc("TRN2", target_bir_lowering=False, debug=False,
                   num_devices=NCORES)

    # host-prearranged layouts (see build_in_maps), all fp16:
    # qd [128 p=(bi,d), (g 16, hh 2, n 256)] fp16, pre-scaled by ALPHA
    # kd [128 p=(bi,d), (g 16, hh 2, mb 2, m 128)] fp16
    # vd [128 p=m, (b 128, c 2, e 33)] fp16 (e==32 -> 1.0)
    qd = nc.dram_tensor("qd", [128, 8192], FP16, kind="ExternalInput")
    kd = nc.dram_tensor("kd", [128, 8192], FP16, kind="ExternalInput")
    vd = nc.dram_tensor("vd", [128, 8448], FP16, kind="ExternalInput")
    biasd_d = nc.dram_tensor("biasd", [2, 1040], F32, kind="ExternalInput")
    b3c_d = nc.dram_tensor("b3c", [8, 1], F32, kind="ExternalInput")
    cbig_d = nc.dram_tensor("cbig", [128, CONSTW], F32, kind="ExternalInput")

    posd = nc.dram_tensor("posd", [1, 1024], F32R, kind="Internal")
    # raw O (32 cols) + Z (1 col) per j, 8 j per half-group, fp16
    out_d = nc.dram_tensor("out", [128, 8448], FP16, kind="ExternalOutput")

    MODES, EMUL_ENG = _tile_modes()

    with TileContext(nc) as tc:
        with (
            tc.tile_pool(name="const", bufs=1) as constp,
            tc.tile_pool(name="vpool", bufs=1) as vpool,
            tc.tile_pool(name="mlp", bufs=2) as mlpp,
            tc.tile_pool(name="epool", bufs=int(os.environ.get("K_EP", "28"))) as epool,
            tc.tile_pool(name="spsum", bufs=int(os.environ.get("K_SB", "3")), space="PSUM") as spsum,
            tc.tile_pool(name="auxpsum", bufs=int(os.environ.get("K_AB", "2")), space="PSUM") as auxpsum,
        ):
            # ---- full-size q/k/v SBUF tiles; chunked loads emitted lazily
            q_all = vpool.tile([128, 8192], FP16)
            k_all = vpool.tile([128, 8192], FP16)
            v_all = vpool.tile([128, 8448], FP16)

            chunk_loaded = [False] * len(CHUNKS)

            def emit_chunk(ci):
                g0, g1 = CHUNKS[ci]
                ng = g1 - g0 + 1
                qk0 = g0
                if ci == 0:
                    qk0 = 1      # group 0 of q/k loaded via the fast path
                # q on Pool SWDGE; k on sync; v on the DVE queue (all three
                # dispatchers run concurrently; HWDGE generator is shared)
                nc.gpsimd.dma_start(
                    q_all[:, 512 * qk0:512 * (g1 + 1)],
                    AP(qd, 512 * qk0,
                       [[8192, 128], [1, 512 * (g1 - qk0 + 1)]]))
                nc.sync.dma_start(
                    k_all[:, 512 * qk0:512 * (g1 + 1)],
                    AP(kd, 512 * qk0,
                       [[8192, 128], [1, 512 * (g1 - qk0 + 1)]]))
                nc.scalar.dma_start(
                    v_all[:, 528 * g0:528 * (g1 + 1)],
                    AP(vd, 528 * g0, [[8448, 128], [1, 528 * ng]]))

            def ensure_chunk(ci):
                if not chunk_loaded[ci]:
                    chunk_loaded[ci] = True
                    emit_chunk(ci)

            # fast path for the very first QK: k via sync HWDGE, q via Pool
            # SWDGE - different dispatchers run concurrently
            nc.sync.dma_start(k_all[:, 0:512],
                              AP(kd, 0, [[8192, 128], [1, 512]]))
            nc.gpsimd.dma_start(q_all[:, 0:512],
                                AP(qd, 0, [[8192, 128], [1, 512]]))

            ensure_chunk(0)
            ensure_chunk(1)

            biasd = constp.tile([2, 1040], F32)
            nc.sync.dma_start(biasd[:, :], biasd_d[:, :])
            cbig = constp.tile([128, CONSTW], F32)
            nc.sync.dma_start(cbig[:, 0:CONSTWA], cbig_d[:, 0:CONSTWA])
            nc.sync.dma_start(cbig[:, CONSTWA:], cbig_d[:, CONSTWA:])
            b3c = constp.tile([8, 1], F32)
            nc.sync.dma_start(b3c[:, :], b3c_d[:, :])
            jmat_r = constp.tile([128, 128], F32R)
            nc.vector.tensor_copy(jmat_r[:, :],
                                  cbig[:, JMAT_OFF:JMAT_OFF + 128])
            magic_t = constp.tile([128, 8], I32)
            nc.vector.memset(magic_t[:, :], 0x5F3759DF)
            # constant zbias tiles: zk for prefix DVE-exp (bias applied
            # later via expb), zkf for post-prefix DVE-exp (bias PE-folded
            # into S; btrev carries -6144 so the const re-centers the phase)
            zk = constp.tile([128, 1024], F32)
            nc.gpsimd.memset(zk[:, :], ZK_CONST)
            zkf = constp.tile([128, 1024], F32)
            nc.gpsimd.memset(zkf[:, :], ZKF_CONST)

            def cb(nm):
                if nm in _CBA:
                    o = _CBA[nm]
                else:
                    o = CONSTWA + _CBB[nm]
                w = 8 if nm == "w3c" else 128
                return cbig[:, o:o + w]

            mlp_env = {}

            def _mlp_layer(x_sb, g_t, beta_t, w_t, linb_t, last=False):
                x3 = x_sb[:, :].rearrange("p (j f) -> p j f", f=16)
                mz = mlpp.tile([128, 8], F32, tag="mz")
                nc.vector.tensor_reduce(mz[:, :], x3, AX.X, ALU.add)
                xc = mlpp.tile([128, 128], F32, tag="xc")
                xc3 = xc[:, :].rearrange("p (j f) -> p j f", f=16)
                # xc' = mz/16 - x  (negated; g tiles are host-negated)
                nc.vector.scalar_tensor_tensor(
                    xc3, mz[:, :].unsqueeze(2).broadcast_to((128, 8, 16)),
                    1.0 / 16.0, x3, ALU.mult, ALU.subtract)
                sq = mlpp.tile([128, 128], F32, tag="sq")
                sq3 = sq[:, :].rearrange("p (j f) -> p j f", f=16)
                nc.vector.tensor_mul(sq3, xc3, xc3)
                vz = mlpp.tile([128, 8], F32, tag="vz")
                nc.vector.tensor_reduce(vz[:, :], sq3, AX.X, ALU.add)
                # rsqrt(v/16 + eps) fully on DVE (bit-magic + 1 Newton
                # step) so ScalarE only ever runs Exp (one act table).
                w = mlpp.tile([128, 8], F32, tag="w")
                nc.vector.tensor_scalar(w[:, :], vz[:, :], 1.0 / 16.0, 1e-5,
                                        ALU.mult, ALU.add)
                sh = mlpp.tile([128, 8], I32, tag="sh")
                nc.vector.tensor_single_scalar(sh[:, :],
                                               w[:, :].bitcast(I32), 1,
                                               ALU.arith_shift_right)
                yi = mlpp.tile([128, 8], I32, tag="yi")
                nc.vector.tensor_sub(yi[:, :], magic_t[:, :], sh[:, :])
                y0 = yi[:, :].bitcast(F32)
                rz = None
                for it in range(int(os.environ.get("K_NEWTON", "1"))):
                    t = mlpp.tile([128, 8], F32, tag=f"nt{it}")
                    nc.vector.tensor_mul(t[:, :], w[:, :], y0)
                    nc.vector.tensor_mul(t[:, :], t[:, :], y0)
                    nc.vector.tensor_scalar(t[:, :], t[:, :], -0.5, 1.5,
                                            ALU.mult, ALU.add)
                    y1 = mlpp.tile([128, 8], F32, tag=f"ny{it}")
                    nc.vector.tensor_mul(y1[:, :], y0, t[:, :])
                    y0 = y1[:, :]
                    rz = y1
                xn = mlpp.tile([128, 128], F32, tag="xn")
                xn3 = xn[:, :].rearrange("p (j f) -> p j f", f=16)
                nc.vector.tensor_mul(
                    xn3, xc3, rz[:, :].unsqueeze(2).broadcast_to((128, 8, 16)))
                y = mlpp.tile([128, 128], F32, tag="y")
                nc.vector.tensor_mul(y[:, :], xn[:, :], g_t[:, :])
                # +beta folded into the transpose (PSUM accumulate of the
                # column-broadcast betaT const); relu folded into the evac
                pt = auxpsum.tile([128, 512], F32, tag="aux2")
                nc.tensor.matmul(pt[:, :128], y[:, :], cb("ident"),
                                 is_transpose=True, start=True, stop=False)
                nc.tensor.matmul(pt[:, :128], cb("ident"), beta_t,
                                 start=False, stop=True)
                yT = mlpp.tile([128, 128], F32, tag="yT")
                nc.vector.tensor_scalar_max(yT[:, :], pt[:, :128], 0.0)
                if last:
                    return yT
                px = auxpsum.tile([128, 512], F32, tag="aux2")
                nc.tensor.matmul(px[:, :128], yT[:, :], w_t)
                xnext = mlpp.tile([128, 128], F32, tag="xnext")
                nc.vector.tensor_add(xnext[:, :], px[:, :128], linb_t)
                return xnext

            def emit_mlp_stage(stage):
                """0=x0, 1..3=LN layers, 4=pos->DRAM->gather (sets btrev)."""
                env = mlp_env
                if stage == 0:
                    px0 = auxpsum.tile([128, 512], F32, tag="aux2")
                    for j in range(8):
                        nc.tensor.matmul(px0[:, 16 * j:16 * j + 16],
                                         biasd[:, 128 * j:128 * j + 128],
                                         biasd[:, 1024:1040])
                    x0 = mlpp.tile([128, 128], F32, tag="x0")
                    nc.vector.tensor_add(x0[:, :], px0[:, :128], cb("bprojt"))
                    env["x0"] = x0
                    return
                if stage == 1:
                    env["x1"] = _mlp_layer(env["x0"], cb("g1t"), cb("lb1t"),
                                           cb("w1"), cb("linb1t"))
                    return
                if stage == 2:
                    env["x2"] = _mlp_layer(env["x1"], cb("g2t"), cb("lb2t"),
                                           cb("w2"), cb("linb2t"))
                    return
                if stage == 3:
                    env["y3T"] = _mlp_layer(env["x2"], cb("g3t"), cb("lb3t"),
                                            None, None, last=True)
                    return
                # stage 4: posT -> DRAM -> Toeplitz gather (reversed m)
                pos_ps = auxpsum.tile([128, 512], F32, tag="aux2")
                nc.tensor.matmul(pos_ps[0:8, :128], cb("w3c"), env["y3T"][:, :])
                pos_sb = constp.tile([8, 128], F32R)
                nc.vector.tensor_scalar_add(pos_sb[:, :], pos_ps[0:8, :128],
                                            b3c[:, 0:1])
                nc.sync.dma_start(AP(posd, 0, [[128, 8], [1, 128]]),
                                  pos_sb[:, :])
                # btrev as one [128, (mbp 2, c 16, e 16)] tile; 8 DMAs of
                # 16 partitions each, 4D source APs, split across HWDGE
                # (sync/vector) and Pool SWDGE dispatchers
                bt = constp.tile([128, 512], F32R)
                gi = 0
                for mbp in range(2):
                    for a in range(8):
                        src = AP(posd, 31 * (8 * mbp + a),
                                 [[1, 16], [31, 16], [1, 16]])
                        dst = bt[16 * a:16 * a + 16,
                                 256 * mbp:256 * mbp + 256].rearrange(
                            "b (c e) -> b c e", e=16)
                        if gi % 16 < GHW:
                            if gi % 2 == 0:
                                nc.sync.dma_start(dst, src)
                            else:
                                nc.scalar.dma_start(dst, src)
                        else:
                            nc.gpsimd.dma_start(dst, src)
                        gi += 1
                env["btrev"] = [bt[:, 0:256], bt[:, 256:512]]

            def emit_expb():
                """expb (fp16, 2^(bias-6)) from btrev."""
                btrev = mlp_env["btrev"]
                pe_ = auxpsum.tile([128, 512], F32, tag="aux2", name="pexpb")
                for mb in range(2):
                    nc.tensor.matmul(pe_[:, 256 * mb:256 * mb + 256],
                                     jmat_r[:, :], btrev[1 - mb])
                expb = constp.tile([128, 512], FP16)
                nc.scalar.activation(expb[:, :], pe_[:, :512], AF.Exp,
                                     scale=ACT_SCALE)
                return expb

            # --- main pipeline over 64 tiles (2 per half-group) ---
            def emit_qk_exp(g, hh, split_exp=False):
                """QK matmuls + exp for half-group (g, hh) -> epair, emuls."""
                ho = 512 * g + 256 * hh
                epair = []
                emuls = []
                for half in range(2):
                    t_idx = 4 * g + 2 * hh + half
                    mode = MODES[t_idx]
                    in_prefix = t_idx < 4 * NPRE + BRIDGE
                    fold = mode == "F" or (mode == "D" and not in_prefix)
                    sp = spsum.tile([128, 1024], F32, tag="S",
                                    name=f"s{g}_{hh}_{half}")
                    for bi2 in range(2):
                        bi = 2 * half + bi2
                        fo = 512 * bi2
                        for mb in range(2):
                            out_ap = sp[:, fo + 256 * mb:fo + 256 * mb + 256]
                            nc.tensor.matmul(
                                out_ap,
                                k_all[32 * bi:32 * bi + 32,
                                      ho + 128 * mb:ho + 128 * mb + 128],
                                q_all[32 * bi:32 * bi + 32, ho:ho + 256],
                                tile_position=(32 * bi, 0),
                                start=True, stop=not fold)
                            if fold:
                                nc.tensor.matmul(
                                    out_ap, jmat_r[:, :],
                                    mlp_env["btrev"][1 - mb],
                                    tile_position=(0, 0),
                                    start=False, stop=True)
                    e = epool.tile([128, 1024], FP16, tag="E",
                                   name=f"e{g}_{hh}_{half}")
                    if mode == "D":
                        zt = zk if in_prefix else zkf
                        nc.vector._custom_dve(
                            exp_op, out=e[:, :].bitcast(U16),
                            in0=sp[:, :], in1=zt[:, :],
                            s0=BMAGIC, s1=EXP_C1, imm2=EXP_C2)
                        if in_prefix:
                            emuls.append((half, EMUL_ENG[t_idx]))
                    else:
                        if split_exp:
                            nc.scalar.activation(e[:, :512], sp[:, :512],
                                                 AF.Exp, scale=ACT_SCALE)
                            nc.scalar.activation(e[:, 512:], sp[:, 512:],
                                                 AF.Exp, scale=ACT_SCALE)
                        else:
                            nc.scalar.activation(e[:, :], sp[:, :], AF.Exp,
                                                 scale=ACT_SCALE)
                        if mode == "A":
                            emuls.append((half, EMUL_ENG[t_idx]))
                    epair.append(e)
                return epair, emuls

            def emit_emul(expb, epair, emuls):
                for half, eng in emuls:
                    e = epair[half]
                    e4 = e[:, :].rearrange("p (j mb n) -> p j mb n",
                                           mb=2, n=256)
                    bb = (expb[:, :].rearrange("p (mb n) -> p mb n", n=256)
                          .unsqueeze(1).broadcast_to((128, 2, 2, 256)))
                    if eng == "pool":
                        nc.gpsimd.tensor_mul(e4, e4, bb)
                    else:
                        nc.vector.tensor_mul(e4, e4, bb)

            evac_ctr = [0]

            def emit_pv(g, hh, epair):
                o_ps = auxpsum.tile([128, 264], F32, tag="aux2",
                                    name=f"ops{g}_{hh}")
                for bi in range(4):
                    e = epair[bi // 2]
                    fo = 512 * (bi % 2)
                    vb = 66 * (8 * g + 4 * hh + bi)
                    for nb in range(2):
                        j = 2 * bi + nb
                        for c in range(2):
                            nc.tensor.matmul(
                                o_ps[:, 33 * j:33 * j + 33],
                                e[:, fo + 256 * c + 128 * nb:
                                  fo + 256 * c + 128 * nb + 128],
                                v_all[:, vb + 33 * c:vb + 33 * c + 33],
                                start=(c == 0), stop=(c == 1))
                return o_ps

            def emit_evac_store(g, hh, o_ps, split=False):
                osb = epool.tile([128, 264], FP16, tag="osb",
                                 name=f"osb{g}_{hh}")

                def one(j0, nj):
                    src_ = o_ps[:, 33 * j0:33 * (j0 + nj)]
                    dst_ = osb[:, 33 * j0:33 * (j0 + nj)]
                    i = evac_ctr[0]
                    evac_ctr[0] += 1
                    if ((i + 1) * EVA_ACT) // 32 > (i * EVA_ACT) // 32:
                        nc.scalar.activation(dst_, src_, AF.Copy)
                    else:
                        nc.vector.tensor_copy(dst_, src_)
                    nc.sync.dma_start(
                        AP(out_d, 264 * (2 * g + hh) + 33 * j0,
                           [[8448, 128], [1, 33 * nj]]),
                        osb[:, 33 * j0:33 * (j0 + nj)])

                if split:
                    one(0, 4)
                    one(4, 4)
                else:
                    one(0, 8)

            # ---- schedule ----
            # Explicit stage lags over half-groups: at hg k the loop emits
            # evac+store(k-3), PV(k-2), emul(k-1), then QK+exp(k), oldest
            # first so each engine's in-order queue sees deps long
            # satisfied.  Prefix only QK+exps (+MLP); backlog drains at
            # DRAIN extra items per stage per new half-group.
            unmul = []   # (g, hh, epair, emuls)  exp'd, bias-mul pending
            unpv = []    # (g, hh, epair)         biased, PV pending
            unev = []    # (g, hh, o_ps)          PV'd, evac+store pending

            def step_evac(n):
                for _ in range(n):
                    if unev:
                        emit_evac_store(*unev.pop(0))

            def step_pv(n):
                for _ in range(n):
                    if unpv:
                        g_, hh_, ep_ = unpv.pop(0)
                        unev.append((g_, hh_, emit_pv(g_, hh_, ep_)))

            def step_emul(n, expb):
                for _ in range(n):
                    if unmul:
                        g_, hh_, ep_, em_ = unmul.pop(0)
                        emit_emul(expb, ep_, em_)
                        unpv.append((g_, hh_, ep_))

            stage_after = {1: 1, 2: 2, 3: 3, 4: 4}  # halfgroup -> mlp stage
            hg = 0
            emit_mlp_stage(0)
            for g in range(NPRE):
                ensure_chunk(CHUNK_OF_GROUP[min(g + 2, NGROUPS - 1)])
                for hh in range(2):
                    ep, em = emit_qk_exp(g, hh, split_exp=(g == 0))
                    unmul.append((g, hh, ep, em))
                    hg += 1
                    st = stage_after.get(hg)
                    if st == 4:
                        # keep Pool's queue ahead of the gathers
                        ensure_chunk(CHUNK_OF_GROUP[4])
                    if st is not None:
                        emit_mlp_stage(st)
            expb = emit_expb()

            CAP = int(os.environ.get("K_CAP", "3"))
            for g in range(NPRE, NGROUPS):
                ensure_chunk(CHUNK_OF_GROUP[min(g + 2, NGROUPS - 1)])
                for hh in range(2):
                    step_evac(min(CAP, max(0, len(unev) - 1)))
                    step_pv(min(CAP, max(0, len(unpv) - 1)))
                    step_emul(min(CAP, max(0, len(unmul) - 1)), expb)
                    ep, em = emit_qk_exp(g, hh)
                    unmul.append((g, hh, ep, em))
            # drain the tail: oldest-first except the final half-group's
            # chain, which jumps the queue (split store for overlap)
            while unmul or unpv or unev:
                step_evac(1)
                step_pv(1)
                step_emul(2, expb)
                if not unmul:
                    step_pv(2)
                    step_evac(2)

    nc.compile()
    _BUILD_CACHE["nc"] = nc
    return nc


def _host_constants():
    hh, ww = 16, 16
    bh, bw = np.meshgrid(np.arange(1 - hh, hh), np.arange(1 - ww, ww),
                         indexing="ij")
    biases = np.stack([bh, bw], -1).reshape(-1, 2).astype(np.float32)
    biasesT = np.zeros((2, 1024), np.float32)
    biasesT[:, :961] = biases.T
    return biasesT


def _blk8(w16):
    cout = w16.shape[1]
    blk = np.zeros((128, 8 * cout), np.float32)
    for j in range(8):
        blk[16 * j:16 * j + 16, cout * j:cout * j + cout] = w16
    return np.ascontiguousarray(blk)


def _tile16(vec):
    return np.ascontiguousarray(
        np.tile(np.asarray(vec, np.float32), (128, 8)))


def _tile16T(vec):
    # column-broadcast (transposed-space) tiling: out[16j+f, r] = vec[f]
    col = np.tile(np.asarray(vec, np.float32), 8)[:, None]
    return np.ascontiguousarray(np.tile(col, (1, 128)))


def build_in_maps(inputs):
    q = np.asarray(inputs["q"], np.float32)
    k = np.asarray(inputs["k"], np.float32)
    v = np.asarray(inputs["v"], np.float32)
    hh = int(np.asarray(inputs["h"]))
    ww = int(np.asarray(inputs["w"]))
    assert hh == 16 and ww == 16, (hh, ww)
    f32 = lambda name: np.asarray(inputs[name], np.float32)
    w3 = f32("w3")
    b3 = f32("b3")

    cblk = {
        "w1": _blk8(f32("w1")), "w2": _blk8(f32("w2")),
        "ident": np.eye(128, dtype=np.float32),
        "bprojt": _tile16(f32("b_proj")),
        "g1t": -_tile16(f32("ln1_g")), "lb1t": _tile16T(f32("ln1_b")),
        "linb1t": _tile16(f32("b1")),
        "g2t": -_tile16(f32("ln2_g")), "lb2t": _tile16T(f32("ln2_b")),
        "linb2t": _tile16(f32("b2")),
        "g3t": -_tile16(f32("ln3_g")), "lb3t": _tile16T(f32("ln3_b")),
    }
    biasd = np.zeros((2, 1040), np.float32)
    biasd[:, 0:1024] = _host_constants()
    biasd[:, 1024:1040] = f32("w_proj")

    def q_layout(x):
        # [128 w, 256 n, 32 d] -> [128 p=(bi,d), (g, hh, n)] fp16, *ALPHA
        x5 = (x * np.float32(ALPHA)).reshape(16, 2, 4, 256, 32)
        return np.ascontiguousarray(
            x5.transpose(2, 4, 0, 1, 3).reshape(128, 8192).astype(np.float16))

    def k_layout(x):
        # [128 w, 256 m, 32 d] -> [128 p=(bi,d), (g, hh, mb, m)] fp16
        x6 = x.reshape(16, 2, 4, 2, 128, 32)        # g hh bi mb m d
        return np.ascontiguousarray(
            x6.transpose(2, 5, 0, 1, 3, 4).reshape(128, 8192)
            .astype(np.float16))

    def v_layout(x):
        # [128 p=m, (b 128, c 2, e 33)] fp16; e==32 -> 1.0
        v4 = x.reshape(128, 2, 128, 32)             # b c p e
        out = np.ones((128, 128, 2, 33), np.float32)
        out[:, :, :, :32] = v4.transpose(2, 0, 1, 3)
        return np.ascontiguousarray(out.reshape(128, 8448).astype(np.float16))

    in_maps = []
    for c in range(NCORES):
        cbig = np.empty((128, CONSTW), np.float32)
        for nm, off in _CBA.items():
            cbig[:, off:off + 128] = cblk[nm]
        for nm, off in _CBB.items():
            if nm == "w3c":
                cbig[:, CONSTWA + off:CONSTWA + off + 8] = _blk8(
                    w3[:, c:c + 1] * np.float32(W3_SCALE))
            else:
                cbig[:, CONSTWA + off:CONSTWA + off + 128] = cblk[nm]
        cbig[:, JMAT_OFF:JMAT_OFF + 128] = np.eye(128, dtype=np.float32)[::-1]
        m = {
            "biasd": biasd,
            "cbig": np.ascontiguousarray(cbig),
            "qd": q_layout(q[:, c]),
            "kd": k_layout(k[:, c]),
            "vd": v_layout(v[:, c]),
            "b3c": np.full((8, 1),
                           b3[c] * np.float32(W3_SCALE) + np.float32(BSHIFT),
                           np.float32),
        }
        in_maps.append(m)
    return in_maps


def unshard_out(raw):
    # raw [128 p, (g 16, hh 2, j 8, e 33)] fp16 -> [B, N, D] f32 (normalize)
    r5 = raw.reshape(128, 16, 2, 8, 33).astype(np.float32)  # p g hh j e
    O = r5[..., :32]
    Z = r5[..., 32]
    out = O / Z[..., None]
    # b = (g*2+hh)*4 + j//2 ; n = (j%2)*128 + p ; d = e
    o6 = out.reshape(128, 16, 2, 4, 2, 32)           # p g hh bi nb e
    return np.ascontiguousarray(
        o6.transpose(1, 2, 3, 4, 0, 5).reshape(128, 256, 32))


def kernel(**inputs):
    from concourse.bass_utils import run_bass_kernel_spmd

    nc = _build()
    in_maps = build_in_maps(inputs)
    res = run_bass_kernel_spmd(nc, in_maps, core_ids=list(range(NCORES)))
    out = np.empty((B, H, N, D), np.float32)
    for c in range(NCORES):
        out[:, c] = unshard_out(res.results[c]["out"])
    return out


# revision 19
# speedup vs baseline: 1.0582x; 1.0118x over previous
"""Windowed attention + dynamic relative position bias on 8 NeuronCores.

Shapes: q,k,v [B=128, H=8, N=256, D=32] f32; pos-MLP width P=16; h=w=16.
Sharding: head-parallel - core c computes head c for all 128 batch windows;
the per-core head is selected purely by the w3 column passed to that core
(program is SPMD-identical).

v2 design (multi-engine exp, fp16 IO, host normalization):
  - All math in "z units": S = alpha*qk with alpha = 1024*log2(e)/sqrt(D),
    bias path in 1024*log2(e) units. exp(x) == 2^(Z/1024) for Z in z units.
  - Softmax exp is split across TWO engines per tile ([128,1024] of S):
      Act: activation Exp (scale=ln2/1024) -> fp16 E.
      DVE: custom fused op EXP2_BITS_ANT computing the BITS of fp16(2^z):
           Z=S+zbias; N=(Z+B)-B (magic round to 1024s); F=Z-N;
           u16 = trunc(C2*F*F + Z + C1).  One DVE instr per tile, bias
           fused via zbias (Src1).  Max rel err ~0.2%.
  - Bias application per Act tile: either PE fold (J-matmul accumulate of
    btrev into S) or post-exp multiply by expb (fp16; on DVE or Pool).
  - No on-device normalization: PV uses ones-augmented V; O and Z columns
    are DMA'd PSUM->DRAM with f32->fp16 conversion; host divides O/Z.
  - q/k/v are host-packed fp16 (q pre-scaled by alpha): halves DMA bytes.
  - Per-row exp scale constants differ per tile path but every softmax row
    lives inside one tile, so constants cancel in the host division.
  - DMA dispatch spread to respect the shared HWDGE generator: q chunks on
    Pool SWDGE, k/consts/outs on sync, v/gathers on the DVE queue.
"""

import os
import numpy as np

B, H, N, D = 128, 8, 256, 32
P = 16
NCORES = 8
NGROUPS = 16
NPRE = int(os.environ.get("K_NPRE", "4"))          # prefix groups before expb
DRAIN = int(os.environ.get("K_DRAIN", "2"))        # extra PVs per new half-group
CHUNKS = [(0, 1), (2, 5), (6, 10), (11, 15)]
CHUNK_OF_GROUP = [0, 0, 1, 1, 1, 1, 2, 2, 2, 2, 2, 3, 3, 3, 3, 3]

LOG2E = 1.4426950408889634
ALPHA = float(1024.0 * LOG2E / np.sqrt(D))         # q prescale (host)
ACT_SCALE = float(np.log(2.0) / 1024.0)            # Act exp scale in z units
W3_SCALE = float(1024.0 * LOG2E)                   # bias MLP output scale
BSHIFT = -6144.0                                   # -6 octaves, inside b3c
BMAGIC = float(1.5 * 2 ** 33)                      # fp32 round-to-1024 magic
EXP_C1 = 433.57                                    # mantissa-parabola const
EXP_C2 = 3.3007e-4                                 # mantissa-parabola curv
ZK_CONST = 12800.0                                 # prefix-DVE zbias (K only)
ZKF_CONST = 12800.0 + 6144.0                       # folded-DVE zbias const

# tile modes: 'A' = Act exp + later emul; 'F' = Act exp + PE fold;
#             'D' = DVE custom exp (fused bias); prefix 'D' uses ZK + emul.
X_CNT = int(os.environ.get("K_X", "18"))           # DVE-exp tiles (of 64)
F_CNT = int(os.environ.get("K_F", "12"))            # PE-fold tiles
MDVE_CNT = int(os.environ.get("K_MDVE", "19"))     # DVE-emul quota (rest Pool)
PRE_D = int(os.environ.get("K_PRED", "0"))         # prefix DVE-exp tiles
BRIDGE = int(os.environ.get("K_BRIDGE", "4"))      # unfolded tiles after prefix
GHW = int(os.environ.get("K_GHW", "10"))           # gathers on HWDGE (of 16)
EVA_ACT = int(os.environ.get("K_EVA", "13"))       # PSUM evacs on Act (of 32)

# merged const block: cbiga | cbigb | jmat
_CBA = {"bprojt": 0, "g1t": 128, "lb1t": 256, "linb1t": 384, "w1": 512}
_CBB = {"w2": 0, "ident": 128, "g2t": 256, "lb2t": 384, "linb2t": 512,
        "g3t": 640, "lb3t": 768, "w3c": 896}
CONSTWA = 640
CONSTWB = 904
CONSTW = CONSTWA + CONSTWB + 128                   # + jmat
JMAT_OFF = CONSTWA + CONSTWB

_BUILD_CACHE = {}


def _tile_modes():
    """Assign one of A/F/D to each of the 64 tiles, plus emul engine."""
    modes = [None] * 64
    npre_t = 4 * NPRE
    dleft = PRE_D
    for t in range(npre_t):
        if dleft > 0 and t % 2 == 1:
            modes[t] = "D"
            dleft -= 1
        else:
            modes[t] = "A"
    npost = 64 - npre_t
    d_post = X_CNT - (PRE_D - dleft)
    acc_d = acc_f = 0
    for i in range(npost):
        t = npre_t + i
        nd = ((i + 1) * d_post) // npost
        if nd > acc_d:
            modes[t] = "D"
            acc_d = nd
        else:
            nf = ((i + 1) * F_CNT) // npost
            if nf > acc_f and t >= npre_t + BRIDGE:
                modes[t] = "F"
                acc_f = nf
            else:
                modes[t] = "A"
    eng = {}
    for t in range(npre_t + BRIDGE):
        eng[t] = "dve"
    post_need = [t for t in range(npre_t + BRIDGE, 64) if modes[t] == "A"]
    nm = max(len(post_need), 1)
    for j, t in enumerate(post_need):
        if ((j + 1) * MDVE_CNT) // nm > (j * MDVE_CNT) // nm:
            eng[t] = "dve"
        else:
            eng[t] = "pool"
    return modes, eng


def _register_exp_op():
    if "op" in _BUILD_CACHE:
        return _BUILD_CACHE["op"]
    from concourse.dve_spec import Spec, Src0, Src1, C0, C1, C2, lower
    from concourse import dve_ops
    from concourse.dve_table_gen import dve_ver_for
    from concourse.dve_uop import DveOpSpec

    for o in dve_ops.OPS:
        if o.name == "EXP2_BITS_ANT":
            _BUILD_CACHE["op"] = o
            return o

    Z = Src0 + Src1
    Nq = (Z + C0) - C0
    F = Z - Nq
    body = (C2 * F) * F + (Z + C1)

    def ref(in0, in1, s0, s1, imm2):
        f32 = np.float32
        Zv = f32(f32(in0.astype(np.float32)) + f32(in1.astype(np.float32)))
        t = f32(Zv + f32(s0))
        Nv = f32(t - f32(s0))
        Fv = f32(Zv - Nv)
        u = f32(f32(f32(f32(imm2)) * Fv) * Fv + f32(Zv + f32(s1)))
        return np.clip(u, 0.0, 65535.0)

    spec = Spec(body=body, reference=ref)
    ver = dve_ver_for("TRN2")
    row = dve_ops._CUSTOM_DVE_ROW_BASE + len(dve_ops.OPS)
    sha = DveOpSpec(name="EXP2_BITS_ANT", opcode=row,
                    uops=lower(spec, ver=ver), rd1_en=True).sha(ver)
    op = dve_ops.DveOp("EXP2_BITS_ANT", spec, subdim=False,
                       uops_sha={ver: sha})
    dve_ops.OPS.append(op)
    dve_ops.CUSTOM_DVE_SPECS[op.name] = spec
    dve_ops._SUB_OPCODE_FOR_NAME[op.name] = row
    _BUILD_CACHE["op"] = op
    return op


def _build():
    if "nc" in _BUILD_CACHE:
        return _BUILD_CACHE["nc"]
    import concourse.bacc as bacc
    import concourse.mybir as mybir
    from concourse.tile import TileContext
    from bass_rust import AP

    exp_op = _register_exp_op()

    F32 = mybir.dt.float32
    F32R = mybir.dt.float32r
    FP16 = mybir.dt.float16
    U16 = mybir.dt.uint16
    AF = mybir.ActivationFunctionType
    AX = mybir.AxisListType
    ALU = mybir.AluOpType
    I32 = mybir.dt.int32

    nc = bacc.Bacc("TRN2", target_bir_lowering=False, debug=False,
                   num_devices=NCORES)

    # host-prearranged layouts (see build_in_maps), all fp16:
    # qd [128 p=(bi,d), (g 16, hh 2, n 256)] fp16, pre-scaled by ALPHA
    # kd [128 p=(bi,d), (g 16, hh 2, mb 2, m 128)] fp16
    # vd [128 p=m, (b 128, c 2, e 33)] fp16 (e==32 -> 1.0)
    qd = nc.dram_tensor("qd", [128, 8192], FP16, kind="ExternalInput")
    kd = nc.dram_tensor("kd", [128, 8192], FP16, kind="ExternalInput")
    vd = nc.dram_tensor("vd", [128, 8448], FP16, kind="ExternalInput")
    biasd_d = nc.dram_tensor("biasd", [2, 1040], F32, kind="ExternalInput")
    b3c_d = nc.dram_tensor("b3c", [8, 1], F32, kind="ExternalInput")
    cbig_d = nc.dram_tensor("cbig", [128, CONSTW], F32, kind="ExternalInput")

    posd = nc.dram_tensor("posd", [1, 1024], F32R, kind="Internal")
    # raw O (32 cols) + Z (1 col) per j, 8 j per half-group, fp16
    out_d = nc.dram_tensor("out", [128, 8448], FP16, kind="ExternalOutput")

    MODES, EMUL_ENG = _tile_modes()

    with TileContext(nc) as tc:
        with (
            tc.tile_pool(name="const", bufs=1) as constp,
            tc.tile_pool(name="vpool", bufs=1) as vpool,
            tc.tile_pool(name="mlp", bufs=2) as mlpp,
            tc.tile_pool(name="epool", bufs=int(os.environ.get("K_EP", "28"))) as epool,
            tc.tile_pool(name="spsum", bufs=int(os.environ.get("K_SB", "3")), space="PSUM") as spsum,
            tc.tile_pool(name="auxpsum", bufs=int(os.environ.get("K_AB", "2")), space="PSUM") as auxpsum,
        ):
            # ---- full-size q/k/v SBUF tiles; chunked loads emitted lazily
            q_all = vpool.tile([128, 8192], FP16)
            k_all = vpool.tile([128, 8192], FP16)
            v_all = vpool.tile([128, 8448], FP16)

            chunk_loaded = [False] * len(CHUNKS)

            def emit_chunk(ci):
                g0, g1 = CHUNKS[ci]
                ng = g1 - g0 + 1
                qk0 = g0
                if ci == 0:
                    qk0 = 1      # group 0 of q/k loaded via the fast path
                # q on Pool SWDGE; k on sync; v on the DVE queue (all three
                # dispatchers run concurrently; HWDGE generator is shared)
                nc.gpsimd.dma_start(
                    q_all[:, 512 * qk0:512 * (g1 + 1)],
                    AP(qd, 512 * qk0,
                       [[8192, 128], [1, 512 * (g1 - qk0 + 1)]]))
                nc.sync.dma_start(
                    k_all[:, 512 * qk0:512 * (g1 + 1)],
                    AP(kd, 512 * qk0,
                       [[8192, 128], [1, 512 * (g1 - qk0 + 1)]]))
                nc.scalar.dma_start(
                    v_all[:, 528 * g0:528 * (g1 + 1)],
                    AP(vd, 528 * g0, [[8448, 128], [1, 528 * ng]]))

            def ensure_chunk(ci):
                if not chunk_loaded[ci]:
                    chunk_loaded[ci] = True
                    emit_chunk(ci)

            # fast path for the very first QK: k via sync HWDGE, q via Pool
            # SWDGE - different dispatchers run concurrently
            nc.sync.dma_start(k_all[:, 0:512],
                              AP(kd, 0, [[8192, 128], [1, 512]]))
            nc.gpsimd.dma_start(q_all[:, 0:512],
                                AP(qd, 0, [[8192, 128], [1, 512]]))

            biasd = constp.tile([2, 1040], F32)
            nc.sync.dma_start(biasd[:, :], biasd_d[:, :])
            cbig = constp.tile([128, CONSTW], F32)
            nc.sync.dma_start(cbig[:, 0:CONSTWA], cbig_d[:, 0:CONSTWA])
            ensure_chunk(0)
            ensure_chunk(1)
            nc.sync.dma_start(cbig[:, CONSTWA:], cbig_d[:, CONSTWA:])
            b3c = constp.tile([8, 1], F32)
            nc.sync.dma_start(b3c[:, :], b3c_d[:, :])
            jmat_r = constp.tile([128, 128], F32R)
            nc.vector.tensor_copy(jmat_r[:, :],
                                  cbig[:, JMAT_OFF:JMAT_OFF + 128])
            magic_t = constp.tile([128, 8], I32)
            nc.vector.memset(magic_t[:, :], 0x5F3759DF)
            # constant zbias tiles: zk for prefix DVE-exp (bias applied
            # later via expb), zkf for post-prefix DVE-exp (bias PE-folded
            # into S; btrev carries -6144 so the const re-centers the phase)
            zk = constp.tile([128, 1024], F32)
            nc.gpsimd.memset(zk[:, :], ZK_CONST)
            zkf = constp.tile([128, 1024], F32)
            nc.gpsimd.memset(zkf[:, :], ZKF_CONST)

            def cb(nm):
                if nm in _CBA:
                    o = _CBA[nm]
                else:
                    o = CONSTWA + _CBB[nm]
                w = 8 if nm == "w3c" else 128
                return cbig[:, o:o + w]

            mlp_env = {}

            def _mlp_layer(x_sb, g_t, beta_t, w_t, linb_t, last=False):
                x3 = x_sb[:, :].rearrange("p (j f) -> p j f", f=16)
                mz = mlpp.tile([128, 8], F32, tag="mz")
                nc.vector.tensor_reduce(mz[:, :], x3, AX.X, ALU.add)
                xc = mlpp.tile([128, 128], F32, tag="xc")
                xc3 = xc[:, :].rearrange("p (j f) -> p j f", f=16)
                # xc' = mz/16 - x  (negated; g tiles are host-negated)
                nc.vector.scalar_tensor_tensor(
                    xc3, mz[:, :].unsqueeze(2).broadcast_to((128, 8, 16)),
                    1.0 / 16.0, x3, ALU.mult, ALU.subtract)
                sq = mlpp.tile([128, 128], F32, tag="sq")
                sq3 = sq[:, :].rearrange("p (j f) -> p j f", f=16)
                nc.vector.tensor_mul(sq3, xc3, xc3)
                vz = mlpp.tile([128, 8], F32, tag="vz")
                nc.vector.tensor_reduce(vz[:, :], sq3, AX.X, ALU.add)
                # rsqrt(v/16 + eps) fully on DVE (bit-magic + 1 Newton
                # step) so ScalarE only ever runs Exp (one act table).
                w = mlpp.tile([128, 8], F32, tag="w")
                nc.vector.tensor_scalar(w[:, :], vz[:, :], 1.0 / 16.0, 1e-5,
                                        ALU.mult, ALU.add)
                sh = mlpp.tile([128, 8], I32, tag="sh")
                nc.vector.tensor_single_scalar(sh[:, :],
                                               w[:, :].bitcast(I32), 1,
                                               ALU.arith_shift_right)
                yi = mlpp.tile([128, 8], I32, tag="yi")
                nc.vector.tensor_sub(yi[:, :], magic_t[:, :], sh[:, :])
                y0 = yi[:, :].bitcast(F32)
                rz = None
                for it in range(int(os.environ.get("K_NEWTON", "1"))):
                    t = mlpp.tile([128, 8], F32, tag=f"nt{it}")
                    nc.vector.tensor_mul(t[:, :], w[:, :], y0)
                    nc.vector.tensor_mul(t[:, :], t[:, :], y0)
                    nc.vector.tensor_scalar(t[:, :], t[:, :], -0.5, 1.5,
                                            ALU.mult, ALU.add)
                    y1 = mlpp.tile([128, 8], F32, tag=f"ny{it}")
                    nc.vector.tensor_mul(y1[:, :], y0, t[:, :])
                    y0 = y1[:, :]
                    rz = y1
                xn = mlpp.tile([128, 128], F32, tag="xn")
                xn3 = xn[:, :].rearrange("p (j f) -> p j f", f=16)
                nc.vector.tensor_mul(
                    xn3, xc3, rz[:, :].unsqueeze(2).broadcast_to((128, 8, 16)))
                y = mlpp.tile([128, 128], F32, tag="y")
                nc.vector.tensor_mul(y[:, :], xn[:, :], g_t[:, :])
                # +beta folded into the transpose (PSUM accumulate of the
                # column-broadcast betaT const); relu folded into the evac
                pt = auxpsum.tile([128, 512], F32, tag="aux2")
                nc.tensor.matmul(pt[:, :128], y[:, :], cb("ident"),
                                 is_transpose=True, start=True, stop=False)
                nc.tensor.matmul(pt[:, :128], cb("ident"), beta_t,
                                 start=False, stop=True)
                yT = mlpp.tile([128, 128], F32, tag="yT")
                nc.vector.tensor_scalar_max(yT[:, :], pt[:, :128], 0.0)
                if last:
                    return yT
                px = auxpsum.tile([128, 512], F32, tag="aux2")
                nc.tensor.matmul(px[:, :128], yT[:, :], w_t)
                xnext = mlpp.tile([128, 128], F32, tag="xnext")
                nc.vector.tensor_add(xnext[:, :], px[:, :128], linb_t)
                return xnext

            def emit_mlp_stage(stage):
                """0=x0, 1..3=LN layers, 4=pos->DRAM->gather (sets btrev)."""
                env = mlp_env
                if stage == 0:
                    px0 = auxpsum.tile([128, 512], F32, tag="aux2")
                    for j in range(8):
                        nc.tensor.matmul(px0[:, 16 * j:16 * j + 16],
                                         biasd[:, 128 * j:128 * j + 128],
                                         biasd[:, 1024:1040])
                    x0 = mlpp.tile([128, 128], F32, tag="x0")
                    nc.vector.tensor_add(x0[:, :], px0[:, :128], cb("bprojt"))
                    env["x0"] = x0
                    return
                if stage == 1:
                    env["x1"] = _mlp_layer(env["x0"], cb("g1t"), cb("lb1t"),
                                           cb("w1"), cb("linb1t"))
                    return
                if stage == 2:
                    env["x2"] = _mlp_layer(env["x1"], cb("g2t"), cb("lb2t"),
                                           cb("w2"), cb("linb2t"))
                    return
                if stage == 3:
                    env["y3T"] = _mlp_layer(env["x2"], cb("g3t"), cb("lb3t"),
                                            None, None, last=True)
                    return
                # stage 4: posT -> DRAM -> Toeplitz gather (reversed m)
                pos_ps = auxpsum.tile([128, 512], F32, tag="aux2")
                nc.tensor.matmul(pos_ps[0:8, :128], cb("w3c"), env["y3T"][:, :])
                pos_sb = constp.tile([8, 128], F32R)
                nc.vector.tensor_scalar_add(pos_sb[:, :], pos_ps[0:8, :128],
                                            b3c[:, 0:1])
                nc.gpsimd.dma_start(AP(posd, 0, [[128, 8], [1, 128]]),
                                    pos_sb[:, :])
                # btrev as one [128, (mbp 2, c 16, e 16)] tile; 8 DMAs of
                # 16 partitions each, 4D source APs, split across HWDGE
                # (sync/vector) and Pool SWDGE dispatchers
                bt = constp.tile([128, 512], F32R)
                gi = 0
                for mbp in range(2):
                    for a in range(8):
                        src = AP(posd, 31 * (8 * mbp + a),
                                 [[1, 16], [31, 16], [1, 16]])
                        dst = bt[16 * a:16 * a + 16,
                                 256 * mbp:256 * mbp + 256].rearrange(
                            "b (c e) -> b c e", e=16)
                        if gi % 16 < GHW:
                            nc.sync.dma_start(dst, src)
                        else:
                            nc.gpsimd.dma_start(dst, src)
                        gi += 1
                env["btrev"] = [bt[:, 0:256], bt[:, 256:512]]

            def emit_expb():
                """expb (fp16, 2^(bias-6)) from btrev."""
                btrev = mlp_env["btrev"]
                pe_ = auxpsum.tile([128, 512], F32, tag="aux2", name="pexpb")
                for mb in range(2):
                    nc.tensor.matmul(pe_[:, 256 * mb:256 * mb + 256],
                                     jmat_r[:, :], btrev[1 - mb])
                expb = constp.tile([128, 512], FP16)
                nc.scalar.activation(expb[:, :], pe_[:, :512], AF.Exp,
                                     scale=ACT_SCALE)
                return expb

            # --- main pipeline over 64 tiles (2 per half-group) ---
            def emit_qk_exp(g, hh, split_exp=False):
                """QK matmuls + exp for half-group (g, hh) -> epair, emuls."""
                ho = 512 * g + 256 * hh
                epair = []
                emuls = []
                for half in range(2):
                    t_idx = 4 * g + 2 * hh + half
                    mode = MODES[t_idx]
                    in_prefix = t_idx < 4 * NPRE + BRIDGE
                    fold = mode == "F" or (mode == "D" and not in_prefix)
                    sp = spsum.tile([128, 1024], F32, tag="S",
                                    name=f"s{g}_{hh}_{half}")
                    for bi2 in range(2):
                        bi = 2 * half + bi2
                        fo = 512 * bi2
                        for mb in range(2):
                            out_ap = sp[:, fo + 256 * mb:fo + 256 * mb + 256]
                            nc.tensor.matmul(
                                out_ap,
                                k_all[32 * bi:32 * bi + 32,
                                      ho + 128 * mb:ho + 128 * mb + 128],
                                q_all[32 * bi:32 * bi + 32, ho:ho + 256],
                                tile_position=(32 * bi, 0),
                                start=True, stop=not fold)
                            if fold:
                                nc.tensor.matmul(
                                    out_ap, jmat_r[:, :],
                                    mlp_env["btrev"][1 - mb],
                                    tile_position=(0, 0),
                                    start=False, stop=True)
                    e = epool.tile([128, 1024], FP16, tag="E",
                                   name=f"e{g}_{hh}_{half}")
                    if mode == "D":
                        zt = zk if in_prefix else zkf
                        nc.vector._custom_dve(
                            exp_op, out=e[:, :].bitcast(U16),
                            in0=sp[:, :], in1=zt[:, :],
                            s0=BMAGIC, s1=EXP_C1, imm2=EXP_C2)
                        if in_prefix:
                            emuls.append((half, EMUL_ENG[t_idx]))
                    else:
                        if split_exp:
                            nc.scalar.activation(e[:, :512], sp[:, :512],
                                                 AF.Exp, scale=ACT_SCALE)
                            nc.scalar.activation(e[:, 512:], sp[:, 512:],
                                                 AF.Exp, scale=ACT_SCALE)
                        else:
                            nc.scalar.activation(e[:, :], sp[:, :], AF.Exp,
                                                 scale=ACT_SCALE)
                        if mode == "A":
                            emuls.append((half, EMUL_ENG[t_idx]))
                    epair.append(e)
                return epair, emuls

            def emit_emul(expb, epair, emuls):
                for half, eng in emuls:
                    e = epair[half]
                    e4 = e[:, :].rearrange("p (j mb n) -> p j mb n",
                                           mb=2, n=256)
                    bb = (expb[:, :].rearrange("p (mb n) -> p mb n", n=256)
                          .unsqueeze(1).broadcast_to((128, 2, 2, 256)))
                    if eng == "pool":
                        nc.gpsimd.tensor_mul(e4, e4, bb)
                    else:
                        nc.vector.tensor_mul(e4, e4, bb)

            evac_ctr = [0]

            def emit_pv(g, hh, epair):
                o_ps = auxpsum.tile([128, 264], F32, tag="aux2",
                                    name=f"ops{g}_{hh}")
                for bi in range(4):
                    e = epair[bi // 2]
                    fo = 512 * (bi % 2)
                    vb = 66 * (8 * g + 4 * hh + bi)
                    for nb in range(2):
                        j = 2 * bi + nb
                        for c in range(2):
                            nc.tensor.matmul(
                                o_ps[:, 33 * j:33 * j + 33],
                                e[:, fo + 256 * c + 128 * nb:
                                  fo + 256 * c + 128 * nb + 128],
                                v_all[:, vb + 33 * c:vb + 33 * c + 33],
                                start=(c == 0), stop=(c == 1))
                return o_ps

            def emit_evac_store(g, hh, o_ps, split=False):
                osb = epool.tile([128, 264], FP16, tag="osb",
                                 name=f"osb{g}_{hh}")

                def one(j0, nj):
                    src_ = o_ps[:, 33 * j0:33 * (j0 + nj)]
                    dst_ = osb[:, 33 * j0:33 * (j0 + nj)]
                    i = evac_ctr[0]
                    evac_ctr[0] += 1
                    if ((i + 1) * EVA_ACT) // 32 > (i * EVA_ACT) // 32:
                        nc.scalar.activation(dst_, src_, AF.Copy)
                    else:
                        nc.vector.tensor_copy(dst_, src_)
                    nc.sync.dma_start(
                        AP(out_d, 264 * (2 * g + hh) + 33 * j0,
                           [[8448, 128], [1, 33 * nj]]),
                        osb[:, 33 * j0:33 * (j0 + nj)])

                if split:
                    one(0, 4)
                    one(4, 4)
                else:
                    one(0, 8)

            # ---- schedule ----
            # Explicit stage lags over half-groups: at hg k the loop emits
            # evac+store(k-3), PV(k-2), emul(k-1), then QK+exp(k), oldest
            # first so each engine's in-order queue sees deps long
            # satisfied.  Prefix only QK+exps (+MLP); backlog drains at
            # DRAIN extra items per stage per new half-group.
            unmul = []   # (g, hh, epair, emuls)  exp'd, bias-mul pending
            unpv = []    # (g, hh, epair)         biased, PV pending
            unev = []    # (g, hh, o_ps)          PV'd, evac+store pending

            def step_evac(n):
                for _ in range(n):
                    if unev:
                        emit_evac_store(*unev.pop(0))

            def step_pv(n):
                for _ in range(n):
                    if unpv:
                        g_, hh_, ep_ = unpv.pop(0)
                        unev.append((g_, hh_, emit_pv(g_, hh_, ep_)))

            def step_emul(n, expb):
                for _ in range(n):
                    if unmul:
                        g_, hh_, ep_, em_ = unmul.pop(0)
                        emit_emul(expb, ep_, em_)
                        unpv.append((g_, hh_, ep_))

            stage_after = {1: 1, 2: 2, 3: 3, 4: 4}  # halfgroup -> mlp stage
            hg = 0
            emit_mlp_stage(0)
            for g in range(NPRE):
                # prefetch hard: all input chunks are dispatched before the
                # stage-4 gathers enter the sync queue
                ensure_chunk(min(g + 1, len(CHUNKS) - 1))
                for hh in range(2):
                    ep, em = emit_qk_exp(g, hh, split_exp=(g == 0))
                    unmul.append((g, hh, ep, em))
                    hg += 1
                    st = stage_after.get(hg)
                    if st is not None:
                        emit_mlp_stage(st)
            expb = emit_expb()

            CAP = int(os.environ.get("K_CAP", "3"))
            for g in range(NPRE, NGROUPS):
                ensure_chunk(CHUNK_OF_GROUP[min(g + 2, NGROUPS - 1)])
                for hh in range(2):
                    step_evac(min(CAP, max(0, len(unev) - 1)))
                    step_pv(min(CAP, max(0, len(unpv) - 1)))
                    step_emul(min(CAP, max(0, len(unmul) - 1)), expb)
                    ep, em = emit_qk_exp(g, hh)
                    unmul.append((g, hh, ep, em))
            # drain the tail: oldest-first except the final half-group's
            # chain, which jumps the queue (split store for overlap)
            while unmul or unpv or unev:
                step_evac(1)
                step_pv(1)
                step_emul(2, expb)
                if not unmul:
                    step_pv(2)
                    step_evac(2)

    nc.compile()
    _BUILD_CACHE["nc"] = nc
    return nc


def _host_constants():
    hh, ww = 16, 16
    bh, bw = np.meshgrid(np.arange(1 - hh, hh), np.arange(1 - ww, ww),
                         indexing="ij")
    biases = np.stack([bh, bw], -1).reshape(-1, 2).astype(np.float32)
    biasesT = np.zeros((2, 1024), np.float32)
    biasesT[:, :961] = biases.T
    return biasesT


def _blk8(w16):
    cout = w16.shape[1]
    blk = np.zeros((128, 8 * cout), np.float32)
    for j in range(8):
        blk[16 * j:16 * j + 16, cout * j:cout * j + cout] = w16
    return np.ascontiguousarray(blk)


def _tile16(vec):
    return np.ascontiguousarray(
        np.tile(np.asarray(vec, np.float32), (128, 8)))


def _tile16T(vec):
    # column-broadcast (transposed-space) tiling: out[16j+f, r] = vec[f]
    col = np.tile(np.asarray(vec, np.float32), 8)[:, None]
    return np.ascontiguousarray(np.tile(col, (1, 128)))


def build_in_maps(inputs):
    q = np.asarray(inputs["q"], np.float32)
    k = np.asarray(inputs["k"], np.float32)
    v = np.asarray(inputs["v"], np.float32)
    hh = int(np.asarray(inputs["h"]))
    ww = int(np.asarray(inputs["w"]))
    assert hh == 16 and ww == 16, (hh, ww)
    f32 = lambda name: np.asarray(inputs[name], np.float32)
    w3 = f32("w3")
    b3 = f32("b3")

    cblk = {
        "w1": _blk8(f32("w1")), "w2": _blk8(f32("w2")),
        "ident": np.eye(128, dtype=np.float32),
        "bprojt": _tile16(f32("b_proj")),
        "g1t": -_tile16(f32("ln1_g")), "lb1t": _tile16T(f32("ln1_b")),
        "linb1t": _tile16(f32("b1")),
        "g2t": -_tile16(f32("ln2_g")), "lb2t": _tile16T(f32("ln2_b")),
        "linb2t": _tile16(f32("b2")),
        "g3t": -_tile16(f32("ln3_g")), "lb3t": _tile16T(f32("ln3_b")),
    }
    biasd = np.zeros((2, 1040), np.float32)
    biasd[:, 0:1024] = _host_constants()
    biasd[:, 1024:1040] = f32("w_proj")

    def q_layout(x):
        # [128 w, 256 n, 32 d] -> [128 p=(bi,d), (g, hh, n)] fp16, *ALPHA
        x5 = (x * np.float32(ALPHA)).reshape(16, 2, 4, 256, 32)
        return np.ascontiguousarray(
            x5.transpose(2, 4, 0, 1, 3).reshape(128, 8192).astype(np.float16))

    def k_layout(x):
        # [128 w, 256 m, 32 d] -> [128 p=(bi,d), (g, hh, mb, m)] fp16
        x6 = x.reshape(16, 2, 4, 2, 128, 32)        # g hh bi mb m d
        return np.ascontiguousarray(
            x6.transpose(2, 5, 0, 1, 3, 4).reshape(128, 8192)
            .astype(np.float16))

    def v_layout(x):
        # [128 p=m, (b 128, c 2, e 33)] fp16; e==32 -> 1.0
        v4 = x.reshape(128, 2, 128, 32)             # b c p e
        out = np.ones((128, 128, 2, 33), np.float32)
        out[:, :, :, :32] = v4.transpose(2, 0, 1, 3)
        return np.ascontiguousarray(out.reshape(128, 8448).astype(np.float16))

    in_maps = []
    for c in range(NCORES):
        cbig = np.empty((128, CONSTW), np.float32)
        for nm, off in _CBA.items():
            cbig[:, off:off + 128] = cblk[nm]
        for nm, off in _CBB.items():
            if nm == "w3c":
                cbig[:, CONSTWA + off:CONSTWA + off + 8] = _blk8(
                    w3[:, c:c + 1] * np.float32(W3_SCALE))
            else:
                cbig[:, CONSTWA + off:CONSTWA + off + 128] = cblk[nm]
        cbig[:, JMAT_OFF:JMAT_OFF + 128] = np.eye(128, dtype=np.float32)[::-1]
        m = {
            "biasd": biasd,
            "cbig": np.ascontiguousarray(cbig),
            "qd": q_layout(q[:, c]),
            "kd": k_layout(k[:, c]),
            "vd": v_layout(v[:, c]),
            "b3c": np.full((8, 1),
                           b3[c] * np.float32(W3_SCALE) + np.float32(BSHIFT),
                           np.float32),
        }
        in_maps.append(m)
    return in_maps


def unshard_out(raw):
    # raw [128 p, (g 16, hh 2, j 8, e 33)] fp16 -> [B, N, D] f32 (normalize)
    r5 = raw.reshape(128, 16, 2, 8, 33).astype(np.float32)  # p g hh j e
    O = r5[..., :32]
    Z = r5[..., 32]
    out = O / Z[..., None]
    # b = (g*2+hh)*4 + j//2 ; n = (j%2)*128 + p ; d = e
    o6 = out.reshape(128, 16, 2, 4, 2, 32)           # p g hh bi nb e
    return np.ascontiguousarray(
        o6.transpose(1, 2, 3, 4, 0, 5).reshape(128, 256, 32))


def kernel(**inputs):
    from concourse.bass_utils import run_bass_kernel_spmd

    nc = _build()
    in_maps = build_in_maps(inputs)
    res = run_bass_kernel_spmd(nc, in_maps, core_ids=list(range(NCORES)))
    out = np.empty((B, H, N, D), np.float32)
    for c in range(NCORES):
        out[:, c] = unshard_out(res.results[c]["out"])
    return out


# revision 20
# speedup vs baseline: 1.0669x; 1.0082x over previous
"""Windowed attention + dynamic relative position bias on 8 NeuronCores.

Shapes: q,k,v [B=128, H=8, N=256, D=32] f32; pos-MLP width P=16; h=w=16.
Sharding: head-parallel - core c computes head c for all 128 batch windows;
the per-core head is selected purely by the w3 column passed to that core
(program is SPMD-identical).

v2 design (multi-engine exp, fp16 IO, host normalization):
  - All math in "z units": S = alpha*qk with alpha = 1024*log2(e)/sqrt(D),
    bias path in 1024*log2(e) units. exp(x) == 2^(Z/1024) for Z in z units.
  - Softmax exp is split across TWO engines per tile ([128,1024] of S):
      Act: activation Exp (scale=ln2/1024) -> fp16 E.
      DVE: custom fused op EXP2_BITS_ANT computing the BITS of fp16(2^z):
           Z=S+zbias; N=(Z+B)-B (magic round to 1024s); F=Z-N;
           u16 = trunc(C2*F*F + Z + C1).  One DVE instr per tile, bias
           fused via zbias (Src1).  Max rel err ~0.2%.
  - Bias application per Act tile: either PE fold (J-matmul accumulate of
    btrev into S) or post-exp multiply by expb (fp16; on DVE or Pool).
  - No on-device normalization: PV uses ones-augmented V; O and Z columns
    are DMA'd PSUM->DRAM with f32->fp16 conversion; host divides O/Z.
  - q/k/v are host-packed fp16 (q pre-scaled by alpha): halves DMA bytes.
  - Per-row exp scale constants differ per tile path but every softmax row
    lives inside one tile, so constants cancel in the host division.
  - DMA dispatch spread to respect the shared HWDGE generator: q chunks on
    Pool SWDGE, k/consts/outs on sync, v/gathers on the DVE queue.
"""

import os
import numpy as np

B, H, N, D = 128, 8, 256, 32
P = 16
NCORES = 8
NGROUPS = 16
NPRE = int(os.environ.get("K_NPRE", "4"))          # prefix groups before expb
DRAIN = int(os.environ.get("K_DRAIN", "2"))        # extra PVs per new half-group
CHUNKS = [(0, 1), (2, 5), (6, 10), (11, 15)]
CHUNK_OF_GROUP = [0, 0, 1, 1, 1, 1, 2, 2, 2, 2, 2, 3, 3, 3, 3, 3]

LOG2E = 1.4426950408889634
ALPHA = float(1024.0 * LOG2E / np.sqrt(D))         # q prescale (host)
ACT_SCALE = float(np.log(2.0) / 1024.0)            # Act exp scale in z units
W3_SCALE = float(1024.0 * LOG2E)                   # bias MLP output scale
BSHIFT = -6144.0                                   # -6 octaves, inside b3c
BMAGIC = float(1.5 * 2 ** 33)                      # fp32 round-to-1024 magic
EXP_C1 = 433.57                                    # mantissa-parabola const
EXP_C2 = 3.3007e-4                                 # mantissa-parabola curv
ZK_CONST = 12800.0                                 # prefix-DVE zbias (K only)
ZKF_CONST = 12800.0 + 6144.0                       # folded-DVE zbias const

# tile modes: 'A' = Act exp + later emul; 'F' = Act exp + PE fold;
#             'D' = DVE custom exp (fused bias); prefix 'D' uses ZK + emul.
X_CNT = int(os.environ.get("K_X", "23"))           # DVE-exp tiles (of 64)
F_CNT = int(os.environ.get("K_F", "10"))            # PE-fold tiles
MDVE_CNT = int(os.environ.get("K_MDVE", "0"))     # DVE-emul quota (rest Pool)
PRE_D = int(os.environ.get("K_PRED", "0"))         # prefix DVE-exp tiles
BRIDGE = int(os.environ.get("K_BRIDGE", "4"))      # unfolded tiles after prefix
GHW = int(os.environ.get("K_GHW", "10"))           # gathers on HWDGE (of 16)
EVA_ACT = int(os.environ.get("K_EVA", "14"))       # PSUM evacs on Act (of 32)

# merged const block: cbiga | cbigb | jmat
_CBA = {"bprojt": 0, "g1t": 128, "lb1t": 256, "linb1t": 384, "w1": 512}
_CBB = {"w2": 0, "ident": 128, "g2t": 256, "lb2t": 384, "linb2t": 512,
        "g3t": 640, "lb3t": 768, "w3c": 896}
CONSTWA = 640
CONSTWB = 904
CONSTW = CONSTWA + CONSTWB + 128                   # + jmat
JMAT_OFF = CONSTWA + CONSTWB

_BUILD_CACHE = {}


def _tile_modes():
    """Assign one of A/F/D to each of the 64 tiles, plus emul engine."""
    modes = [None] * 64
    npre_t = 4 * NPRE
    dleft = PRE_D
    for t in range(npre_t):
        if dleft > 0 and t % 2 == 1:
            modes[t] = "D"
            dleft -= 1
        else:
            modes[t] = "A"
    npost = 64 - npre_t
    d_post = X_CNT - (PRE_D - dleft)
    acc_d = acc_f = 0
    for i in range(npost):
        t = npre_t + i
        nd = ((i + 1) * d_post) // npost
        if nd > acc_d:
            modes[t] = "D"
            acc_d = nd
        else:
            nf = ((i + 1) * F_CNT) // npost
            if nf > acc_f and t >= npre_t + BRIDGE:
                modes[t] = "F"
                acc_f = nf
            else:
                modes[t] = "A"
    eng = {}
    for t in range(npre_t + BRIDGE):
        eng[t] = "dve"
    post_need = [t for t in range(npre_t + BRIDGE, 64) if modes[t] == "A"]
    nm = max(len(post_need), 1)
    for j, t in enumerate(post_need):
        if ((j + 1) * MDVE_CNT) // nm > (j * MDVE_CNT) // nm:
            eng[t] = "dve"
        else:
            eng[t] = "pool"
    return modes, eng


def _register_exp_op():
    if "op" in _BUILD_CACHE:
        return _BUILD_CACHE["op"]
    from concourse.dve_spec import Spec, Src0, Src1, C0, C1, C2, lower
    from concourse import dve_ops
    from concourse.dve_table_gen import dve_ver_for
    from concourse.dve_uop import DveOpSpec

    for o in dve_ops.OPS:
        if o.name == "EXP2_BITS_ANT":
            _BUILD_CACHE["op"] = o
            return o

    Z = Src0 + Src1
    Nq = (Z + C0) - C0
    F = Z - Nq
    body = (C2 * F) * F + (Z + C1)

    def ref(in0, in1, s0, s1, imm2):
        f32 = np.float32
        Zv = f32(f32(in0.astype(np.float32)) + f32(in1.astype(np.float32)))
        t = f32(Zv + f32(s0))
        Nv = f32(t - f32(s0))
        Fv = f32(Zv - Nv)
        u = f32(f32(f32(f32(imm2)) * Fv) * Fv + f32(Zv + f32(s1)))
        return np.clip(u, 0.0, 65535.0)

    spec = Spec(body=body, reference=ref)
    ver = dve_ver_for("TRN2")
    row = dve_ops._CUSTOM_DVE_ROW_BASE + len(dve_ops.OPS)
    sha = DveOpSpec(name="EXP2_BITS_ANT", opcode=row,
                    uops=lower(spec, ver=ver), rd1_en=True).sha(ver)
    op = dve_ops.DveOp("EXP2_BITS_ANT", spec, subdim=False,
                       uops_sha={ver: sha})
    dve_ops.OPS.append(op)
    dve_ops.CUSTOM_DVE_SPECS[op.name] = spec
    dve_ops._SUB_OPCODE_FOR_NAME[op.name] = row
    _BUILD_CACHE["op"] = op
    return op


def _build():
    if "nc" in _BUILD_CACHE:
        return _BUILD_CACHE["nc"]
    import concourse.bacc as bacc
    import concourse.mybir as mybir
    from concourse.tile import TileContext
    from bass_rust import AP

    exp_op = _register_exp_op()

    F32 = mybir.dt.float32
    F32R = mybir.dt.float32r
    FP16 = mybir.dt.float16
    U16 = mybir.dt.uint16
    AF = mybir.ActivationFunctionType
    AX = mybir.AxisListType
    ALU = mybir.AluOpType
    I32 = mybir.dt.int32

    nc = bacc.Bacc("TRN2", target_bir_lowering=False, debug=False,
                   num_devices=NCORES)

    # host-prearranged layouts (see build_in_maps), all fp16:
    # qd [128 p=(bi,d), (g 16, hh 2, n 256)] fp16, pre-scaled by ALPHA
    # kd [128 p=(bi,d), (g 16, hh 2, mb 2, m 128)] fp16
    # vd [128 p=m, (b 128, c 2, e 33)] fp16 (e==32 -> 1.0)
    qd = nc.dram_tensor("qd", [128, 8192], FP16, kind="ExternalInput")
    kd = nc.dram_tensor("kd", [128, 8192], FP16, kind="ExternalInput")
    vd = nc.dram_tensor("vd", [128, 8448], FP16, kind="ExternalInput")
    biasd_d = nc.dram_tensor("biasd", [2, 1040], F32, kind="ExternalInput")
    b3c_d = nc.dram_tensor("b3c", [8, 1], F32, kind="ExternalInput")
    cbig_d = nc.dram_tensor("cbig", [128, CONSTW], F32, kind="ExternalInput")

    posd = nc.dram_tensor("posd", [1, 1024], F32R, kind="Internal")
    # raw O (32 cols) + Z (1 col) per j, 8 j per half-group, fp16
    out_d = nc.dram_tensor("out", [128, 8448], FP16, kind="ExternalOutput")

    MODES, EMUL_ENG = _tile_modes()

    with TileContext(nc) as tc:
        with (
            tc.tile_pool(name="const", bufs=1) as constp,
            tc.tile_pool(name="vpool", bufs=1) as vpool,
            tc.tile_pool(name="mlp", bufs=2) as mlpp,
            tc.tile_pool(name="epool", bufs=int(os.environ.get("K_EP", "28"))) as epool,
            tc.tile_pool(name="spsum", bufs=int(os.environ.get("K_SB", "3")), space="PSUM") as spsum,
            tc.tile_pool(name="auxpsum", bufs=int(os.environ.get("K_AB", "2")), space="PSUM") as auxpsum,
        ):
            # ---- full-size q/k/v SBUF tiles; chunked loads emitted lazily
            q_all = vpool.tile([128, 8192], FP16)
            k_all = vpool.tile([128, 8192], FP16)
            v_all = vpool.tile([128, 8448], FP16)

            chunk_loaded = [False] * len(CHUNKS)

            def emit_chunk(ci):
                g0, g1 = CHUNKS[ci]
                ng = g1 - g0 + 1
                qk0 = g0
                if ci == 0:
                    qk0 = 1      # group 0 of q/k loaded via the fast path
                # q on Pool SWDGE; k on sync; v on the DVE queue (all three
                # dispatchers run concurrently; HWDGE generator is shared)
                nc.gpsimd.dma_start(
                    q_all[:, 512 * qk0:512 * (g1 + 1)],
                    AP(qd, 512 * qk0,
                       [[8192, 128], [1, 512 * (g1 - qk0 + 1)]]))
                nc.sync.dma_start(
                    k_all[:, 512 * qk0:512 * (g1 + 1)],
                    AP(kd, 512 * qk0,
                       [[8192, 128], [1, 512 * (g1 - qk0 + 1)]]))
                nc.scalar.dma_start(
                    v_all[:, 528 * g0:528 * (g1 + 1)],
                    AP(vd, 528 * g0, [[8448, 128], [1, 528 * ng]]))

            def ensure_chunk(ci):
                if not chunk_loaded[ci]:
                    chunk_loaded[ci] = True
                    emit_chunk(ci)

            # fast path for the very first QK: k via sync HWDGE, q via Pool
            # SWDGE - different dispatchers run concurrently
            nc.sync.dma_start(k_all[:, 0:512],
                              AP(kd, 0, [[8192, 128], [1, 512]]))
            nc.gpsimd.dma_start(q_all[:, 0:512],
                                AP(qd, 0, [[8192, 128], [1, 512]]))

            biasd = constp.tile([2, 1040], F32)
            nc.sync.dma_start(biasd[:, :], biasd_d[:, :])
            cbig = constp.tile([128, CONSTW], F32)
            nc.sync.dma_start(cbig[:, 0:CONSTWA], cbig_d[:, 0:CONSTWA])
            ensure_chunk(0)
            ensure_chunk(1)
            nc.sync.dma_start(cbig[:, CONSTWA:], cbig_d[:, CONSTWA:])
            b3c = constp.tile([8, 1], F32)
            nc.sync.dma_start(b3c[:, :], b3c_d[:, :])
            jmat_r = constp.tile([128, 128], F32R)
            nc.vector.tensor_copy(jmat_r[:, :],
                                  cbig[:, JMAT_OFF:JMAT_OFF + 128])
            magic_t = constp.tile([128, 8], I32)
            nc.vector.memset(magic_t[:, :], 0x5F3759DF)
            # constant zbias tiles: zk for prefix DVE-exp (bias applied
            # later via expb), zkf for post-prefix DVE-exp (bias PE-folded
            # into S; btrev carries -6144 so the const re-centers the phase)
            zk = constp.tile([128, 1024], F32)
            nc.gpsimd.memset(zk[:, :], ZK_CONST)
            zkf = constp.tile([128, 1024], F32)
            nc.gpsimd.memset(zkf[:, :], ZKF_CONST)

            def cb(nm):
                if nm in _CBA:
                    o = _CBA[nm]
                else:
                    o = CONSTWA + _CBB[nm]
                w = 8 if nm == "w3c" else 128
                return cbig[:, o:o + w]

            mlp_env = {}

            def _mlp_layer(x_sb, g_t, beta_t, w_t, linb_t, last=False):
                x3 = x_sb[:, :].rearrange("p (j f) -> p j f", f=16)
                mz = mlpp.tile([128, 8], F32, tag="mz")
                nc.vector.tensor_reduce(mz[:, :], x3, AX.X, ALU.add)
                xc = mlpp.tile([128, 128], F32, tag="xc")
                xc3 = xc[:, :].rearrange("p (j f) -> p j f", f=16)
                # xc' = mz/16 - x  (negated; g tiles are host-negated)
                nc.vector.scalar_tensor_tensor(
                    xc3, mz[:, :].unsqueeze(2).broadcast_to((128, 8, 16)),
                    1.0 / 16.0, x3, ALU.mult, ALU.subtract)
                sq = mlpp.tile([128, 128], F32, tag="sq")
                sq3 = sq[:, :].rearrange("p (j f) -> p j f", f=16)
                nc.vector.tensor_mul(sq3, xc3, xc3)
                vz = mlpp.tile([128, 8], F32, tag="vz")
                nc.vector.tensor_reduce(vz[:, :], sq3, AX.X, ALU.add)
                # rsqrt(v/16 + eps) fully on DVE (bit-magic + 1 Newton
                # step) so ScalarE only ever runs Exp (one act table).
                w = mlpp.tile([128, 8], F32, tag="w")
                nc.vector.tensor_scalar(w[:, :], vz[:, :], 1.0 / 16.0, 1e-5,
                                        ALU.mult, ALU.add)
                sh = mlpp.tile([128, 8], I32, tag="sh")
                nc.vector.tensor_single_scalar(sh[:, :],
                                               w[:, :].bitcast(I32), 1,
                                               ALU.arith_shift_right)
                yi = mlpp.tile([128, 8], I32, tag="yi")
                nc.vector.tensor_sub(yi[:, :], magic_t[:, :], sh[:, :])
                y0 = yi[:, :].bitcast(F32)
                rz = None
                for it in range(int(os.environ.get("K_NEWTON", "1"))):
                    t = mlpp.tile([128, 8], F32, tag=f"nt{it}")
                    nc.vector.tensor_mul(t[:, :], w[:, :], y0)
                    nc.vector.tensor_mul(t[:, :], t[:, :], y0)
                    nc.vector.tensor_scalar(t[:, :], t[:, :], -0.5, 1.5,
                                            ALU.mult, ALU.add)
                    y1 = mlpp.tile([128, 8], F32, tag=f"ny{it}")
                    nc.vector.tensor_mul(y1[:, :], y0, t[:, :])
                    y0 = y1[:, :]
                    rz = y1
                xn = mlpp.tile([128, 128], F32, tag="xn")
                xn3 = xn[:, :].rearrange("p (j f) -> p j f", f=16)
                nc.vector.tensor_mul(
                    xn3, xc3, rz[:, :].unsqueeze(2).broadcast_to((128, 8, 16)))
                y = mlpp.tile([128, 128], F32, tag="y")
                nc.vector.tensor_mul(y[:, :], xn[:, :], g_t[:, :])
                # +beta folded into the transpose (PSUM accumulate of the
                # column-broadcast betaT const); relu folded into the evac
                pt = auxpsum.tile([128, 512], F32, tag="aux2")
                nc.tensor.matmul(pt[:, :128], y[:, :], cb("ident"),
                                 is_transpose=True, start=True, stop=False)
                nc.tensor.matmul(pt[:, :128], cb("ident"), beta_t,
                                 start=False, stop=True)
                yT = mlpp.tile([128, 128], F32, tag="yT")
                nc.vector.tensor_scalar_max(yT[:, :], pt[:, :128], 0.0)
                if last:
                    return yT
                px = auxpsum.tile([128, 512], F32, tag="aux2")
                nc.tensor.matmul(px[:, :128], yT[:, :], w_t)
                xnext = mlpp.tile([128, 128], F32, tag="xnext")
                nc.vector.tensor_add(xnext[:, :], px[:, :128], linb_t)
                return xnext

            def emit_mlp_stage(stage):
                """0=x0, 1..3=LN layers, 4=pos->DRAM->gather (sets btrev)."""
                env = mlp_env
                if stage == 0:
                    px0 = auxpsum.tile([128, 512], F32, tag="aux2")
                    for j in range(8):
                        nc.tensor.matmul(px0[:, 16 * j:16 * j + 16],
                                         biasd[:, 128 * j:128 * j + 128],
                                         biasd[:, 1024:1040])
                    x0 = mlpp.tile([128, 128], F32, tag="x0")
                    nc.vector.tensor_add(x0[:, :], px0[:, :128], cb("bprojt"))
                    env["x0"] = x0
                    return
                if stage == 1:
                    env["x1"] = _mlp_layer(env["x0"], cb("g1t"), cb("lb1t"),
                                           cb("w1"), cb("linb1t"))
                    return
                if stage == 2:
                    env["x2"] = _mlp_layer(env["x1"], cb("g2t"), cb("lb2t"),
                                           cb("w2"), cb("linb2t"))
                    return
                if stage == 3:
                    env["y3T"] = _mlp_layer(env["x2"], cb("g3t"), cb("lb3t"),
                                            None, None, last=True)
                    return
                # stage 4: posT -> DRAM -> Toeplitz gather (reversed m)
                pos_ps = auxpsum.tile([128, 512], F32, tag="aux2")
                nc.tensor.matmul(pos_ps[0:8, :128], cb("w3c"), env["y3T"][:, :])
                pos_sb = constp.tile([8, 128], F32R)
                nc.vector.tensor_scalar_add(pos_sb[:, :], pos_ps[0:8, :128],
                                            b3c[:, 0:1])
                nc.gpsimd.dma_start(AP(posd, 0, [[128, 8], [1, 128]]),
                                    pos_sb[:, :])
                # btrev as one [128, (mbp 2, c 16, e 16)] tile; 8 DMAs of
                # 16 partitions each, 4D source APs, split across HWDGE
                # (sync/vector) and Pool SWDGE dispatchers
                bt = constp.tile([128, 512], F32R)
                gi = 0
                for mbp in range(2):
                    for a in range(8):
                        src = AP(posd, 31 * (8 * mbp + a),
                                 [[1, 16], [31, 16], [1, 16]])
                        dst = bt[16 * a:16 * a + 16,
                                 256 * mbp:256 * mbp + 256].rearrange(
                            "b (c e) -> b c e", e=16)
                        if gi % 16 < GHW:
                            nc.sync.dma_start(dst, src)
                        else:
                            nc.gpsimd.dma_start(dst, src)
                        gi += 1
                env["btrev"] = [bt[:, 0:256], bt[:, 256:512]]

            def emit_expb():
                """expb (fp16, 2^(bias-6)) from btrev."""
                btrev = mlp_env["btrev"]
                pe_ = auxpsum.tile([128, 512], F32, tag="aux2", name="pexpb")
                for mb in range(2):
                    nc.tensor.matmul(pe_[:, 256 * mb:256 * mb + 256],
                                     jmat_r[:, :], btrev[1 - mb])
                expb = constp.tile([128, 512], FP16)
                nc.scalar.activation(expb[:, :], pe_[:, :512], AF.Exp,
                                     scale=ACT_SCALE)
                return expb

            # --- main pipeline over 64 tiles (2 per half-group) ---
            def emit_qk_exp(g, hh, split_exp=False):
                """QK matmuls + exp for half-group (g, hh) -> epair, emuls."""
                ho = 512 * g + 256 * hh
                epair = []
                emuls = []
                for half in range(2):
                    t_idx = 4 * g + 2 * hh + half
                    mode = MODES[t_idx]
                    in_prefix = t_idx < 4 * NPRE + BRIDGE
                    fold = mode == "F" or (mode == "D" and not in_prefix)
                    sp = spsum.tile([128, 1024], F32, tag="S",
                                    name=f"s{g}_{hh}_{half}")
                    for bi2 in range(2):
                        bi = 2 * half + bi2
                        fo = 512 * bi2
                        for mb in range(2):
                            out_ap = sp[:, fo + 256 * mb:fo + 256 * mb + 256]
                            nc.tensor.matmul(
                                out_ap,
                                k_all[32 * bi:32 * bi + 32,
                                      ho + 128 * mb:ho + 128 * mb + 128],
                                q_all[32 * bi:32 * bi + 32, ho:ho + 256],
                                tile_position=(32 * bi, 0),
                                start=True, stop=not fold)
                            if fold:
                                nc.tensor.matmul(
                                    out_ap, jmat_r[:, :],
                                    mlp_env["btrev"][1 - mb],
                                    tile_position=(0, 0),
                                    start=False, stop=True)
                    e = epool.tile([128, 1024], FP16, tag="E",
                                   name=f"e{g}_{hh}_{half}")
                    if mode == "D":
                        zt = zk if in_prefix else zkf
                        nc.vector._custom_dve(
                            exp_op, out=e[:, :].bitcast(U16),
                            in0=sp[:, :], in1=zt[:, :],
                            s0=BMAGIC, s1=EXP_C1, imm2=EXP_C2)
                        if in_prefix:
                            emuls.append((half, EMUL_ENG[t_idx]))
                    else:
                        if split_exp:
                            nc.scalar.activation(e[:, :512], sp[:, :512],
                                                 AF.Exp, scale=ACT_SCALE)
                            nc.scalar.activation(e[:, 512:], sp[:, 512:],
                                                 AF.Exp, scale=ACT_SCALE)
                        else:
                            nc.scalar.activation(e[:, :], sp[:, :], AF.Exp,
                                                 scale=ACT_SCALE)
                        if mode == "A":
                            emuls.append((half, EMUL_ENG[t_idx]))
                    epair.append(e)
                return epair, emuls

            def emit_emul(expb, epair, emuls):
                for half, eng in emuls:
                    e = epair[half]
                    e4 = e[:, :].rearrange("p (j mb n) -> p j mb n",
                                           mb=2, n=256)
                    bb = (expb[:, :].rearrange("p (mb n) -> p mb n", n=256)
                          .unsqueeze(1).broadcast_to((128, 2, 2, 256)))
                    if eng == "pool":
                        nc.gpsimd.tensor_mul(e4, e4, bb)
                    else:
                        nc.vector.tensor_mul(e4, e4, bb)

            evac_ctr = [0]

            def emit_pv(g, hh, epair):
                o_ps = auxpsum.tile([128, 264], F32, tag="aux2",
                                    name=f"ops{g}_{hh}")
                for bi in range(4):
                    e = epair[bi // 2]
                    fo = 512 * (bi % 2)
                    vb = 66 * (8 * g + 4 * hh + bi)
                    for nb in range(2):
                        j = 2 * bi + nb
                        for c in range(2):
                            nc.tensor.matmul(
                                o_ps[:, 33 * j:33 * j + 33],
                                e[:, fo + 256 * c + 128 * nb:
                                  fo + 256 * c + 128 * nb + 128],
                                v_all[:, vb + 33 * c:vb + 33 * c + 33],
                                start=(c == 0), stop=(c == 1))
                return o_ps

            def emit_evac_store(g, hh, o_ps, split=False):
                osb = epool.tile([128, 264], FP16, tag="osb",
                                 name=f"osb{g}_{hh}")

                def one(j0, nj):
                    src_ = o_ps[:, 33 * j0:33 * (j0 + nj)]
                    dst_ = osb[:, 33 * j0:33 * (j0 + nj)]
                    i = evac_ctr[0]
                    evac_ctr[0] += 1
                    if ((i + 1) * EVA_ACT) // 32 > (i * EVA_ACT) // 32:
                        nc.scalar.activation(dst_, src_, AF.Copy)
                    else:
                        nc.vector.tensor_copy(dst_, src_)
                    nc.sync.dma_start(
                        AP(out_d, 264 * (2 * g + hh) + 33 * j0,
                           [[8448, 128], [1, 33 * nj]]),
                        osb[:, 33 * j0:33 * (j0 + nj)])

                if split:
                    one(0, 4)
                    one(4, 4)
                else:
                    one(0, 8)

            # ---- schedule ----
            # Explicit stage lags over half-groups: at hg k the loop emits
            # evac+store(k-3), PV(k-2), emul(k-1), then QK+exp(k), oldest
            # first so each engine's in-order queue sees deps long
            # satisfied.  Prefix only QK+exps (+MLP); backlog drains at
            # DRAIN extra items per stage per new half-group.
            unmul = []   # (g, hh, epair, emuls)  exp'd, bias-mul pending
            unpv = []    # (g, hh, epair)         biased, PV pending
            unev = []    # (g, hh, o_ps)          PV'd, evac+store pending

            def step_evac(n):
                for _ in range(n):
                    if unev:
                        emit_evac_store(*unev.pop(0))

            def step_pv(n):
                for _ in range(n):
                    if unpv:
                        g_, hh_, ep_ = unpv.pop(0)
                        unev.append((g_, hh_, emit_pv(g_, hh_, ep_)))

            def step_emul(n, expb):
                for _ in range(n):
                    if unmul:
                        g_, hh_, ep_, em_ = unmul.pop(0)
                        emit_emul(expb, ep_, em_)
                        unpv.append((g_, hh_, ep_))

            stage_after = {1: 1, 2: 2, 3: 3, 4: 4}  # halfgroup -> mlp stage
            hg = 0
            emit_mlp_stage(0)
            for g in range(NPRE):
                # prefetch hard: all input chunks are dispatched before the
                # stage-4 gathers enter the sync queue
                ensure_chunk(min(g + 1, len(CHUNKS) - 1))
                for hh in range(2):
                    ep, em = emit_qk_exp(g, hh, split_exp=(g == 0))
                    unmul.append((g, hh, ep, em))
                    hg += 1
                    st = stage_after.get(hg)
                    if st is not None:
                        emit_mlp_stage(st)
            expb = emit_expb()

            CAP = int(os.environ.get("K_CAP", "3"))
            for g in range(NPRE, NGROUPS):
                ensure_chunk(CHUNK_OF_GROUP[min(g + 2, NGROUPS - 1)])
                for hh in range(2):
                    step_evac(min(CAP, max(0, len(unev) - 1)))
                    step_pv(min(CAP, max(0, len(unpv) - 1)))
                    step_emul(min(CAP, max(0, len(unmul) - 1)), expb)
                    ep, em = emit_qk_exp(g, hh)
                    unmul.append((g, hh, ep, em))
            # drain the tail: oldest-first except the final half-group's
            # chain, which jumps the queue (split store for overlap)
            while unmul or unpv or unev:
                step_evac(1)
                step_pv(1)
                step_emul(2, expb)
                if not unmul:
                    step_pv(2)
                    step_evac(2)

    nc.compile()
    _BUILD_CACHE["nc"] = nc
    return nc


def _host_constants():
    hh, ww = 16, 16
    bh, bw = np.meshgrid(np.arange(1 - hh, hh), np.arange(1 - ww, ww),
                         indexing="ij")
    biases = np.stack([bh, bw], -1).reshape(-1, 2).astype(np.float32)
    biasesT = np.zeros((2, 1024), np.float32)
    biasesT[:, :961] = biases.T
    return biasesT


def _blk8(w16):
    cout = w16.shape[1]
    blk = np.zeros((128, 8 * cout), np.float32)
    for j in range(8):
        blk[16 * j:16 * j + 16, cout * j:cout * j + cout] = w16
    return np.ascontiguousarray(blk)


def _tile16(vec):
    return np.ascontiguousarray(
        np.tile(np.asarray(vec, np.float32), (128, 8)))


def _tile16T(vec):
    # column-broadcast (transposed-space) tiling: out[16j+f, r] = vec[f]
    col = np.tile(np.asarray(vec, np.float32), 8)[:, None]
    return np.ascontiguousarray(np.tile(col, (1, 128)))


def build_in_maps(inputs):
    q = np.asarray(inputs["q"], np.float32)
    k = np.asarray(inputs["k"], np.float32)
    v = np.asarray(inputs["v"], np.float32)
    hh = int(np.asarray(inputs["h"]))
    ww = int(np.asarray(inputs["w"]))
    assert hh == 16 and ww == 16, (hh, ww)
    f32 = lambda name: np.asarray(inputs[name], np.float32)
    w3 = f32("w3")
    b3 = f32("b3")

    cblk = {
        "w1": _blk8(f32("w1")), "w2": _blk8(f32("w2")),
        "ident": np.eye(128, dtype=np.float32),
        "bprojt": _tile16(f32("b_proj")),
        "g1t": -_tile16(f32("ln1_g")), "lb1t": _tile16T(f32("ln1_b")),
        "linb1t": _tile16(f32("b1")),
        "g2t": -_tile16(f32("ln2_g")), "lb2t": _tile16T(f32("ln2_b")),
        "linb2t": _tile16(f32("b2")),
        "g3t": -_tile16(f32("ln3_g")), "lb3t": _tile16T(f32("ln3_b")),
    }
    biasd = np.zeros((2, 1040), np.float32)
    biasd[:, 0:1024] = _host_constants()
    biasd[:, 1024:1040] = f32("w_proj")

    def q_layout(x):
        # [128 w, 256 n, 32 d] -> [128 p=(bi,d), (g, hh, n)] fp16, *ALPHA
        x5 = (x * np.float32(ALPHA)).reshape(16, 2, 4, 256, 32)
        return np.ascontiguousarray(
            x5.transpose(2, 4, 0, 1, 3).reshape(128, 8192).astype(np.float16))

    def k_layout(x):
        # [128 w, 256 m, 32 d] -> [128 p=(bi,d), (g, hh, mb, m)] fp16
        x6 = x.reshape(16, 2, 4, 2, 128, 32)        # g hh bi mb m d
        return np.ascontiguousarray(
            x6.transpose(2, 5, 0, 1, 3, 4).reshape(128, 8192)
            .astype(np.float16))

    def v_layout(x):
        # [128 p=m, (b 128, c 2, e 33)] fp16; e==32 -> 1.0
        v4 = x.reshape(128, 2, 128, 32)             # b c p e
        out = np.ones((128, 128, 2, 33), np.float32)
        out[:, :, :, :32] = v4.transpose(2, 0, 1, 3)
        return np.ascontiguousarray(out.reshape(128, 8448).astype(np.float16))

    in_maps = []
    for c in range(NCORES):
        cbig = np.empty((128, CONSTW), np.float32)
        for nm, off in _CBA.items():
            cbig[:, off:off + 128] = cblk[nm]
        for nm, off in _CBB.items():
            if nm == "w3c":
                cbig[:, CONSTWA + off:CONSTWA + off + 8] = _blk8(
                    w3[:, c:c + 1] * np.float32(W3_SCALE))
            else:
                cbig[:, CONSTWA + off:CONSTWA + off + 128] = cblk[nm]
        cbig[:, JMAT_OFF:JMAT_OFF + 128] = np.eye(128, dtype=np.float32)[::-1]
        m = {
            "biasd": biasd,
            "cbig": np.ascontiguousarray(cbig),
            "qd": q_layout(q[:, c]),
            "kd": k_layout(k[:, c]),
            "vd": v_layout(v[:, c]),
            "b3c": np.full((8, 1),
                           b3[c] * np.float32(W3_SCALE) + np.float32(BSHIFT),
                           np.float32),
        }
        in_maps.append(m)
    return in_maps


def unshard_out(raw):
    # raw [128 p, (g 16, hh 2, j 8, e 33)] fp16 -> [B, N, D] f32 (normalize)
    r5 = raw.reshape(128, 16, 2, 8, 33).astype(np.float32)  # p g hh j e
    O = r5[..., :32]
    Z = r5[..., 32]
    out = O / Z[..., None]
    # b = (g*2+hh)*4 + j//2 ; n = (j%2)*128 + p ; d = e
    o6 = out.reshape(128, 16, 2, 4, 2, 32)           # p g hh bi nb e
    return np.ascontiguousarray(
        o6.transpose(1, 2, 3, 4, 0, 5).reshape(128, 256, 32))


def kernel(**inputs):
    from concourse.bass_utils import run_bass_kernel_spmd

    nc = _build()
    in_maps = build_in_maps(inputs)
    res = run_bass_kernel_spmd(nc, in_maps, core_ids=list(range(NCORES)))
    out = np.empty((B, H, N, D), np.float32)
    for c in range(NCORES):
        out[:, c] = unshard_out(res.results[c]["out"])
    return out
